# revision 10
# baseline (speedup 1.0000x reference)
"""NeuralFieldCosmo GNN message-passing kernel for 8 Trainium2 NeuronCores.

Math (see reference):
  per-edge MLP on hood_coords: 3 -> 32 -> LN -> relu -> 32 -> LN -> relu
  -> 1024 -> tanh = per-edge weight matrix w[e, 32, 32]
  oc[e, o] = sum_i features[source[e], i] * w[e, o, i]
  out[v] = mean over edges with target[e] == v  (count-clamped)

Sharding: edges sorted by target; core k owns nodes [1280k, 1280k+1280).
Each core runs 10 node-blocks of 128 nodes; block edges padded to S
subtiles of 128 edges. The feature gather is one bulk SWDGE dma_gather.
Scatter-mean is a one-hot matmul accumulated in PSUM per block; counts
are sharding metadata computed on host as reciprocals. Matmuls run in
fp32r (tf32-like). Output slices are concatenated on host.
"""
import numpy as np

import concourse.bass as bass
import concourse.mybir as mybir
import concourse.tile as tile
from concourse import library_config
from concourse.bass_utils import run_bass_kernel_spmd
from concourse.tile_rust import add_dep_helper

P = 128
IN_C = 32
OUT_C = 32
FC = 32
DIM = 3
OI = IN_C * OUT_C  # 1024
FPAD = 64               # feature rows padded to 64 floats (256B) for dma_gather
LN_EPS = 1e-5
RADIUS = 1.0
N_NODES = 10000
N_CORES = 8
V_CORE = 1280           # nodes per core (10 blocks of 128)
NBLK = V_CORE // P      # 10
DVE_OGROUPS = 16        # einsum mul split: o-groups on DVE, rest GPSIMD
F32 = mybir.dt.float32
F32R = mybir.dt.float32r
I16 = mybir.dt.int16
AX = mybir.AxisListType
OP = mybir.AluOpType
AF = mybir.ActivationFunctionType


# ---------------------------------------------------------------- legalize --
def _legalize_bir_json(bir: bytes) -> bytes:
    """This walrus build allows one sync wait per instruction; move excess
    waits onto NoOps inserted before the offending instruction."""
    import orjson
    j = orjson.loads(bir)
    ctr = 0
    for fn in j.get("functions", []):
        for blk in fn.get("blocks", []):
            out = []
            for ins in blk["instructions"]:
                si = ins.get("sync_info")
                waits = (si or {}).get("on_wait") or []
                if len(waits) > 1:
                    for w in waits[:-1]:
                        out.append({
                            "engine": ins.get("engine"), "ins": [],
                            "name": f"legalize-nop-{ctr}", "opcode": "NoOp",
                            "outs": [],
                            "sync_info": {"on_update": [], "on_wait": [w]},
                        })
                        ctr += 1
                    si["on_wait"] = [waits[-1]]
                out.append(ins)
            blk["instructions"] = out
    return orjson.dumps(j)


def _patch_bass(nc):
    orig = nc.to_json_bytes
    nc.to_json_bytes = lambda: _legalize_bir_json(orig())
    return nc


# ------------------------------------------------------------ device kernel --
def _o_view(ap, og0, og1):
    """[128, og*32 : og1*32] viewed as [128, og1-og0, 32]."""
    return ap[:, og0 * 32:og1 * 32].rearrange("p (o i) -> p o i", i=32)


def _r(ap):
    return ap.bitcast(F32R)


def build_kernel(S: int):
    S_tot = NBLK * S
    E_c = S_tot * P

    nc = bass.Bass()
    xaug_d = nc.dram_tensor("xaug", [DIM + 1, E_c], F32, kind="ExternalInput")
    src_d = nc.dram_tensor("srcT", [P, S_tot], mybir.dt.int32, kind="ExternalInput")
    tgt_d = nc.dram_tensor("tgtloc", [P, S_tot], F32, kind="ExternalInput")
    feat_d = nc.dram_tensor("feat", [N_NODES, IN_C], F32, kind="ExternalInput")
    w0_d = nc.dram_tensor("W0aug", [DIM + 1, FC], F32, kind="ExternalInput")
    w1_d = nc.dram_tensor("W1aug", [FC + 1, FC], F32, kind="ExternalInput")
    w2_d = nc.dram_tensor("W2aug", [FC + 1, OI], F32, kind="ExternalInput")
    ln_d = nc.dram_tensor("lnrow", [1, 4 * FC], F32, kind="ExternalInput")
    iota_d = nc.dram_tensor("iotarow", [1, P], F32, kind="ExternalInput")
    ident_d = nc.dram_tensor("ident", [P, P], F32, kind="ExternalInput")
    rcp_d = nc.dram_tensor("rcp", [P, NBLK], F32, kind="ExternalInput")
    out_d = nc.dram_tensor("out", [V_CORE, OUT_C], F32, kind="ExternalOutput")

    groups = []
    s0 = 0
    while s0 < S_tot:
        g = min(8, S_tot - s0)
        groups.append((s0, g))
        s0 += g

    gpsimd_deps = []   # (from_inst, to_inst) ordering for library correctness

    with tile.TileContext(nc) as tc:
        with (
            tc.tile_pool(name="const", bufs=1) as cp,
            tc.tile_pool(name="arrays", bufs=1) as arr,
            tc.tile_pool(name="work", bufs=3) as wk,
        ):
            # ---------------- preload constants ----------------
            tgt_sb = arr.tile([P, S_tot], F32)
            nc.sync.dma_start(out=tgt_sb[:], in_=tgt_d[:])
            rcp_sb = cp.tile([P, NBLK], F32)
            nc.sync.dma_start(out=rcp_sb[:], in_=rcp_d[:])
            identr = cp.tile([P, P], F32)
            nc.sync.dma_start(out=identr[:], in_=ident_d[:])
            identf = cp.tile([P, P], F32R)
            nc.vector.tensor_copy(out=identf[:], in_=identr[:])

            src_sb = arr.tile([P, S_tot], mybir.dt.int32)
            nc.sync.dma_start(out=src_sb[:], in_=src_d[:])

            def gps_dep(inst):
                return inst

            w0r = cp.tile([DIM + 1, FC], F32)
            nc.sync.dma_start(out=w0r[:], in_=w0_d[:])
            w0f = cp.tile([DIM + 1, FC], F32)
            nc.vector.tensor_copy(out=w0f[:], in_=w0r[:])
            w1r = cp.tile([FC + 1, FC], F32)
            nc.sync.dma_start(out=w1r[:], in_=w1_d[:])
            w1f = cp.tile([FC + 1, FC], F32R)
            nc.vector.tensor_copy(out=w1f[:], in_=w1r[:])
            w2r = cp.tile([FC + 1, OI], F32)
            nc.sync.dma_start(out=w2r[:], in_=w2_d[:])
            w2f = cp.tile([FC + 1, OI], F32R)
            nc.vector.tensor_copy(out=w2f[:], in_=w2r[:])

            lnr = cp.tile([1, 4 * FC], F32)
            nc.sync.dma_start(out=lnr[:], in_=ln_d[:])
            iotar = cp.tile([1, P], F32)
            nc.sync.dma_start(out=iotar[:], in_=iota_d[:])
            combo = cp.tile([1, 4 * FC + P], F32)
            nc.vector.tensor_copy(out=combo[:, :4 * FC], in_=lnr[:])
            nc.vector.tensor_copy(out=combo[:, 4 * FC:], in_=iotar[:])
            ones_row = cp.tile([1, P], F32)
            nc.vector.memset(ones_row[:], 1.0)
            eps_t = cp.tile([P, 1], F32)
            nc.vector.memset(eps_t[:], LN_EPS)

            with tc.tile_pool(name="pre_ps", bufs=1, space="PSUM") as pps:
                bc_ps = pps.tile([P, 4 * FC + P], F32, space="PSUM")
                nc.tensor.matmul(out=bc_ps[:], lhsT=ones_row[:], rhs=combo[:],
                                 start=True, stop=True)
                bc = cp.tile([P, 4 * FC + P], F32)
                nc.vector.tensor_copy(out=bc[:], in_=bc_ps[:])
            g0b = bc[:, 0:FC]
            b0b = bc[:, FC:2 * FC]
            g1b = bc[:, 2 * FC:3 * FC]
            b1b = bc[:, 3 * FC:4 * FC]
            iotab = bc[:, 4 * FC:]

            # persistent aug-transpose tiles (row FC is the bias/ones row)
            h0nT = cp.tile([FC + 1, P], F32R)
            nc.vector.tensor_copy(out=h0nT[FC:FC + 1, :], in_=ones_row[:])
            h1nT = cp.tile([FC + 1, P], F32R)
            nc.vector.tensor_copy(out=h1nT[FC:FC + 1, :], in_=ones_row[:])

            h0_all = arr.tile([P, FC * S_tot], F32R)
            h1_all = arr.tile([P, FC * S_tot], F32R)
            s_sum = arr.tile([P, S_tot], F32)
            s_sq = arr.tile([P, S_tot], F32)
            mu0 = arr.tile([P, S_tot], F32)
            rv0 = arr.tile([P, S_tot], F32)
            s1sum = arr.tile([P, S_tot], F32)
            s1sq = arr.tile([P, S_tot], F32)
            mu1 = arr.tile([P, S_tot], F32)
            rv1 = arr.tile([P, S_tot], F32)
            msq = arr.tile([P, S_tot], F32)

            # ---------------- phase 1: L0 + LN0 stats ----------------
            with tc.tile_pool(name="ps1", bufs=2, space="PSUM") as ps1:
                for (g0s, gn) in groups:
                    h0ps = ps1.tile([P, FC * 8], F32, space="PSUM", tag="h0g")
                    hsq = wk.tile([P, FC * 8], F32, tag="hsq")
                    xt = wk.tile([DIM + 1, P * 8], F32, tag="xt")
                    nc.sync.dma_start(out=xt[:, :P * gn],
                                      in_=xaug_d[:, P * g0s:P * (g0s + gn)])
                    for sl in range(gn):
                        s = g0s + sl
                        nc.tensor.matmul(
                            out=h0ps[:, FC * sl:FC * (sl + 1)],
                            lhsT=xt[:, P * sl:P * (sl + 1)],
                            rhs=w0f[:], start=True, stop=True)
                        nc.scalar.activation(
                            out=hsq[:, FC * sl:FC * (sl + 1)],
                            in_=h0ps[:, FC * sl:FC * (sl + 1)], func=AF.Square)
                    w = FC * gn
                    nc.vector.tensor_copy(
                        out=h0_all[:, FC * g0s:FC * (g0s + gn)], in_=h0ps[:, :w])
                    nc.vector.tensor_reduce(
                        out=s_sum[:, g0s:g0s + gn],
                        in_=h0ps[:, :w].rearrange("p (a i) -> p a i", i=FC),
                        axis=AX.X, op=OP.add)
                    nc.vector.tensor_reduce(
                        out=s_sq[:, g0s:g0s + gn],
                        in_=hsq[:, :w].rearrange("p (a i) -> p a i", i=FC),
                        axis=AX.X, op=OP.add)

            # ---------------- phase 2: LN small ops (batched) ----------------
            def ln_smalls(ssum, ssq, mu, rv):
                nc.vector.tensor_scalar(out=mu[:], in0=ssum[:], scalar1=1.0 / FC,
                                        scalar2=None, op0=OP.mult)
                nc.vector.tensor_scalar(out=ssq[:], in0=ssq[:], scalar1=1.0 / FC,
                                        scalar2=None, op0=OP.mult)
                nc.vector.tensor_tensor(out=msq[:], in0=mu[:], in1=mu[:],
                                        op=OP.mult)
                nc.vector.tensor_tensor(out=ssq[:], in0=ssq[:], in1=msq[:],
                                        op=OP.subtract)
                nc.scalar.activation(out=ssq[:], in_=ssq[:], func=AF.Sqrt,
                                     bias=eps_t[:, 0:1])
                nc.vector.reciprocal(out=rv[:], in_=ssq[:])

            ln_smalls(s_sum, s_sq, mu0, rv0)

            # ---------------- phase 3: normalize (in place) ----------------
            def normalize(h_all, mu, rv, gb, bb):
                for gi, (g0s, gn) in enumerate(groups):
                    use_gps = gi % 3 == 2
                    eng = nc.gpsimd if use_gps else nc.vector
                    hv = h_all[:, FC * g0s:FC * (g0s + gn)].rearrange(
                        "p (a i) -> p a i", i=FC)
                    mub = mu[:, g0s:g0s + gn].rearrange(
                        "p (a i) -> p a i", i=1).to_broadcast([P, gn, FC])
                    rvb = rv[:, g0s:g0s + gn].rearrange(
                        "p (a i) -> p a i", i=1).to_broadcast([P, gn, FC])
                    gbt = gb.rearrange("p (a i) -> p a i", a=1).to_broadcast(
                        [P, gn, FC])
                    bbt = bb.rearrange("p (a i) -> p a i", a=1).to_broadcast(
                        [P, gn, FC])
                    ops = [
                        eng.tensor_tensor(out=hv, in0=hv, in1=mub, op=OP.subtract),
                        eng.tensor_tensor(out=hv, in0=hv, in1=rvb, op=OP.mult),
                        eng.tensor_tensor(out=hv, in0=hv, in1=gbt, op=OP.mult),
                        eng.tensor_tensor(out=hv, in0=hv, in1=bbt, op=OP.add),
                    ]
                    relu = eng.tensor_scalar(
                        out=h_all[:, FC * g0s:FC * (g0s + gn)],
                        in0=h_all[:, FC * g0s:FC * (g0s + gn)],
                        scalar1=0.0, scalar2=None, op0=OP.max)
                    if use_gps:
                        for o in ops + [relu]:
                            gps_dep(o)

            normalize(h0_all, mu0, rv0, g0b, b0b)

            # ---------------- phase 4: transpose + L1 + LN1 stats ----------------
            with tc.tile_pool(name="ps4", bufs=2, space="PSUM") as ps4:
                for (g0s, gn) in groups:
                    h1ps = ps4.tile([P, FC * 8], F32, space="PSUM", tag="h1g")
                    hsq = wk.tile([P, FC * 8], F32, tag="hsq")
                    for sl in range(gn):
                        s = g0s + sl
                        t0ps = ps4.tile([FC, P], F32R, space="PSUM", tag="t0")
                        nc.tensor.transpose(
                            out=t0ps[:], in_=h0_all[:, FC * s:FC * (s + 1)],
                            identity=identf[:])
                        nc.scalar.copy(out=h0nT[:FC, :], in_=t0ps[:])
                        nc.tensor.matmul(
                            out=h1ps[:, FC * sl:FC * (sl + 1)],
                            lhsT=h0nT[:], rhs=w1f[:], start=True, stop=True)
                        nc.scalar.activation(
                            out=hsq[:, FC * sl:FC * (sl + 1)],
                            in_=h1ps[:, FC * sl:FC * (sl + 1)], func=AF.Square)
                    w = FC * gn
                    nc.vector.tensor_copy(
                        out=h1_all[:, FC * g0s:FC * (g0s + gn)], in_=h1ps[:, :w])
                    nc.vector.tensor_reduce(
                        out=s1sum[:, g0s:g0s + gn],
                        in_=h1ps[:, :w].rearrange("p (a i) -> p a i", i=FC),
                        axis=AX.X, op=OP.add)
                    nc.vector.tensor_reduce(
                        out=s1sq[:, g0s:g0s + gn],
                        in_=hsq[:, :w].rearrange("p (a i) -> p a i", i=FC),
                        axis=AX.X, op=OP.add)

            # ---------------- phases 5+6: LN1 ----------------
            ln_smalls(s1sum, s1sq, mu1, rv1)
            normalize(h1_all, mu1, rv1, g1b, b1b)

            # ---------------- phase 7: L2 + tanh + mul + scatter ----------------
            with tc.tile_pool(name="ps7", bufs=1, space="PSUM") as psA, \
                 tc.tile_pool(name="ps7b", bufs=2, space="PSUM") as psB:
                for b in range(NBLK):
                    acc = psA.tile([P, OI], F32, space="PSUM", tag="acc")
                    for si in range(S):
                        s = b * S + si
                        oh = wk.tile([P, P], F32R, tag="oh")
                        nc.vector.tensor_tensor(
                            out=oh[:],
                            in0=tgt_sb[:, s:s + 1].to_broadcast([P, P]),
                            in1=iotab, op=OP.is_equal)
                        t1ps = psB.tile([FC, P], F32R, space="PSUM", tag="t1")
                        nc.tensor.transpose(
                            out=t1ps[:], in_=h1_all[:, FC * s:FC * (s + 1)],
                            identity=identf[:])
                        nc.vector.tensor_copy(out=h1nT[:FC, :], in_=t1ps[:])
                        zps = psB.tile([P, OI], F32, space="PSUM", tag="z")
                        nc.tensor.matmul(out=zps[:, 0:512], lhsT=h1nT[:],
                                         rhs=w2f[:, 0:512], start=True, stop=True)
                        nc.tensor.matmul(out=zps[:, 512:OI], lhsT=h1nT[:],
                                         rhs=w2f[:, 512:OI], start=True, stop=True)
                        t = wk.tile([P, OI], F32, tag="t")
                        nc.scalar.activation(out=t[:], in_=zps[:], func=AF.Tanh)
                        f = wk.tile([P, IN_C], F32, tag="f")
                        nc.gpsimd.indirect_dma_start(
                            out=f[:], out_offset=None, in_=feat_d[:],
                            in_offset=bass.IndirectOffsetOnAxis(
                                ap=src_sb[:, s:s + 1], axis=0))
                        prod = wk.tile([P, OI], F32R, tag="prod")
                        fb = f[:].rearrange("p (a i) -> p a i", a=1)
                        dg = DVE_OGROUPS
                        nc.vector.tensor_tensor(
                            out=_o_view(prod, 0, dg), in0=_o_view(t, 0, dg),
                            in1=fb.to_broadcast([P, dg, IN_C]), op=OP.mult)
                        gps_dep(nc.gpsimd.tensor_tensor(
                            out=_o_view(prod, dg, OUT_C), in0=_o_view(t, dg, OUT_C),
                            in1=fb.to_broadcast([P, OUT_C - dg, IN_C]), op=OP.mult))
                        nc.tensor.matmul(out=acc[:, 0:512], lhsT=oh[:],
                                         rhs=prod[:, 0:512], start=(si == 0),
                                         stop=(si == S - 1))
                        nc.tensor.matmul(out=acc[:, 512:OI], lhsT=oh[:],
                                         rhs=prod[:, 512:OI], start=(si == 0),
                                         stop=(si == S - 1))
                    ocv = wk.tile([P, OUT_C], F32, tag="ocv")
                    nc.vector.tensor_reduce(
                        out=ocv[:], in_=acc[:].rearrange("p (o i) -> p o i", i=IN_C),
                        axis=AX.X, op=OP.add)
                    osb = wk.tile([P, OUT_C], F32, tag="osb")
                    nc.vector.tensor_scalar(
                        out=osb[:], in0=ocv[:], scalar1=rcp_sb[:, b:b + 1],
                        scalar2=None, op0=OP.mult)
                    nc.sync.dma_start(out=out_d[P * b:P * (b + 1), :], in_=osb[:])

        for frm, to in gpsimd_deps:
            # add_dep_helper(A, B) == "A waits on B"
            add_dep_helper(to.ins, frm.ins, sync=False, reason="gpsimd library order")

    _patch_bass(nc)
    return nc


# ------------------------------------------------------------- host wrapper --
_cache = {}


def _get_kernel(S):
    if S not in _cache:
        _cache[S] = build_kernel(S)
    return _cache[S]


def kernel(features, hood_coords, source, target,
           W0, b0, g0, beta0, W1, b1, g1, beta1, W2, b2):
    features = np.asarray(features, dtype=np.float32)
    hood = np.asarray(hood_coords, dtype=np.float32)
    source = np.asarray(source).astype(np.int64)
    target = np.asarray(target).astype(np.int64)
    W0 = np.asarray(W0, dtype=np.float32)
    W1 = np.asarray(W1, dtype=np.float32)
    W2 = np.asarray(W2, dtype=np.float32)
    b0 = np.asarray(b0, dtype=np.float32)
    b1 = np.asarray(b1, dtype=np.float32)
    b2 = np.asarray(b2, dtype=np.float32)
    g0 = np.asarray(g0, dtype=np.float32)
    g1 = np.asarray(g1, dtype=np.float32)
    beta0 = np.asarray(beta0, dtype=np.float32)
    beta1 = np.asarray(beta1, dtype=np.float32)

    perm = np.argsort(target, kind="stable")
    tgt_s = target[perm]
    src_s = source[perm]
    hood_s = hood[perm]

    blk_starts = np.searchsorted(tgt_s, np.arange(0, N_CORES * V_CORE + 1, P))
    nseg = np.diff(blk_starts)
    S = max(1, int(np.ceil(nseg.max() / P)))
    S_tot = NBLK * S
    E_c = S_tot * P

    counts = np.bincount(target.astype(np.int64),
                         minlength=N_CORES * V_CORE).astype(np.float32)
    rcp_full = 1.0 / np.maximum(counts, 1.0)


    W0aug = np.vstack([W0 / RADIUS, b0[None, :]]).astype(np.float32)
    W1aug = np.vstack([W1, b1[None, :]]).astype(np.float32)
    W2aug = np.vstack([W2, b2[None, :]]).astype(np.float32)
    lnrow = np.concatenate([g0, beta0, g1, beta1])[None, :].astype(np.float32)
    iotarow = np.arange(P, dtype=np.float32)[None, :]
    ident = np.eye(P, dtype=np.float32)

    in_maps = []
    for k in range(N_CORES):
        xflat = np.zeros((E_c, DIM + 1), dtype=np.float32)
        xflat[:, DIM] = 1.0
        srcflat = np.zeros(E_c, dtype=np.int32)
        tgtflat = np.full(E_c, -1.0, dtype=np.float32)
        for b in range(NBLK):
            gi = k * NBLK + b
            e0, e1 = blk_starts[gi], blk_starts[gi + 1]
            n = e1 - e0
            if n == 0:
                continue
            p0 = b * S * P
            xflat[p0:p0 + n, :DIM] = hood_s[e0:e1]
            srcflat[p0:p0 + n] = src_s[e0:e1].astype(np.int32)
            tgtflat[p0:p0 + n] = (tgt_s[e0:e1] - (k * V_CORE + b * P)).astype(np.float32)
        rcp_k = rcp_full[k * V_CORE:(k + 1) * V_CORE].reshape(NBLK, P).T.copy()
        in_maps.append({
            "xaug": np.ascontiguousarray(xflat.T),
            "srcT": np.ascontiguousarray(srcflat.reshape(S_tot, P).T),
            "tgtloc": np.ascontiguousarray(tgtflat.reshape(S_tot, P).T),
            "feat": features,
            "W0aug": W0aug, "W1aug": W1aug, "W2aug": W2aug,
            "lnrow": lnrow, "iotarow": iotarow, "ident": ident,
            "rcp": np.ascontiguousarray(rcp_k),
        })

    nc = _get_kernel(S)
    res = run_bass_kernel_spmd(nc, in_maps, core_ids=list(range(N_CORES)))

    out = np.zeros((N_NODES, OUT_C), dtype=np.float32)
    for k in range(N_CORES):
        lo = k * V_CORE
        hi = min(lo + V_CORE, N_NODES)
        out[lo:hi] = res.results[k]["out"][:hi - lo]
    return out


# revision 14
# speedup vs baseline: 1.2979x; 1.2979x over previous
"""NeuralFieldCosmo GNN message-passing kernel for 8 Trainium2 NeuronCores.

Math (see reference):
  per-edge MLP on hood_coords: 3 -> 32 -> LN -> relu -> 32 -> LN -> relu
  -> 1024 -> tanh = per-edge weight matrix w[e, 32, 32]
  oc[e, o] = sum_i features[source[e], i] * w[e, o, i]
  out[v] = mean over edges with target[e] == v  (count-clamped)

Sharding: edges sorted by target; core k owns nodes [1280k, 1280k+1280).
Each core runs 10 node-blocks of 128 nodes; block edges padded to S
subtiles of 128 edges. Features are gathered per subtile by indirect DMA.
Scatter-mean is a one-hot matmul accumulated in PSUM per block; counts
are sharding metadata computed on host as reciprocals. Matmuls run in
fp32r (tf32-like). Output slices are concatenated on host.
"""
import numpy as np

import concourse.bass as bass
import concourse.mybir as mybir
import concourse.tile as tile
from concourse import library_config
from concourse.bass_utils import run_bass_kernel_spmd
from concourse.tile_rust import add_dep_helper

P = 128
IN_C = 32
OUT_C = 32
FC = 32
DIM = 3
OI = IN_C * OUT_C  # 1024
FPAD = 64               # feature rows padded to 64 floats (256B) for dma_gather
LN_EPS = 1e-5
RADIUS = 1.0
N_NODES = 10000
N_CORES = 8
V_CORE = 1280           # nodes per core (10 blocks of 128)
NBLK = V_CORE // P      # 10
DVE_OGROUPS = 26        # einsum mul split: o-groups on DVE, rest GPSIMD
F32 = mybir.dt.float32
F32R = mybir.dt.float32r
I16 = mybir.dt.int16
AX = mybir.AxisListType
OP = mybir.AluOpType
AF = mybir.ActivationFunctionType


# ---------------------------------------------------------------- legalize --
def _legalize_bir_json(bir: bytes) -> bytes:
    """This walrus build allows one sync wait per instruction; move excess
    waits onto NoOps inserted before the offending instruction."""
    import orjson
    j = orjson.loads(bir)
    ctr = 0
    for fn in j.get("functions", []):
        for blk in fn.get("blocks", []):
            out = []
            for ins in blk["instructions"]:
                si = ins.get("sync_info")
                waits = (si or {}).get("on_wait") or []
                if len(waits) > 1:
                    for w in waits[:-1]:
                        out.append({
                            "engine": ins.get("engine"), "ins": [],
                            "name": f"legalize-nop-{ctr}", "opcode": "NoOp",
                            "outs": [],
                            "sync_info": {"on_update": [], "on_wait": [w]},
                        })
                        ctr += 1
                    si["on_wait"] = [waits[-1]]
                out.append(ins)
            blk["instructions"] = out
    return orjson.dumps(j)


def _patch_bass(nc):
    orig = nc.to_json_bytes
    nc.to_json_bytes = lambda: _legalize_bir_json(orig())
    return nc


# ------------------------------------------------------------ device kernel --
def _o_view(ap, og0, og1):
    """[128, og*32 : og1*32] viewed as [128, og1-og0, 32]."""
    return ap[:, og0 * 32:og1 * 32].rearrange("p (o i) -> p o i", i=32)


def _r(ap):
    return ap.bitcast(F32R)


def build_kernel(S: int):
    S_tot = NBLK * S
    E_c = S_tot * P

    nc = bass.Bass()
    xaug_d = nc.dram_tensor("xaug", [DIM + 1, E_c], F32, kind="ExternalInput")
    src_d = nc.dram_tensor("srcT", [P, S_tot], mybir.dt.int32, kind="ExternalInput")
    tgt_d = nc.dram_tensor("tgtloc", [P, S_tot], F32, kind="ExternalInput")
    feat_d = nc.dram_tensor("feat", [N_NODES, IN_C], F32, kind="ExternalInput")
    w0_d = nc.dram_tensor("W0aug", [DIM + 1, FC], F32, kind="ExternalInput")
    w1_d = nc.dram_tensor("W1aug", [FC + 1, FC], F32, kind="ExternalInput")
    w2_d = nc.dram_tensor("W2aug", [FC + 1, OI], F32, kind="ExternalInput")
    ln_d = nc.dram_tensor("lnrow", [1, 4 * FC], F32, kind="ExternalInput")
    iota_d = nc.dram_tensor("iotarow", [1, P], F32, kind="ExternalInput")
    ident_d = nc.dram_tensor("ident", [P, P], F32, kind="ExternalInput")
    rcp_d = nc.dram_tensor("rcp", [P, NBLK], F32, kind="ExternalInput")
    out_d = nc.dram_tensor("out", [V_CORE, OUT_C], F32, kind="ExternalOutput")

    groups = []
    s0 = 0
    while s0 < S_tot:
        g = min(8, S_tot - s0)
        groups.append((s0, g))
        s0 += g

    gpsimd_deps = []   # (from_inst, to_inst) ordering for library correctness

    with tile.TileContext(nc) as tc:
        with (
            tc.tile_pool(name="const", bufs=1) as cp,
            tc.tile_pool(name="arrays", bufs=1) as arr,
            tc.tile_pool(name="work", bufs=3) as wk,
        ):
            # ---------------- preload constants ----------------
            tgt_sb = arr.tile([P, S_tot], F32)
            nc.sync.dma_start(out=tgt_sb[:], in_=tgt_d[:])
            rcp_sb = cp.tile([P, NBLK], F32)
            nc.sync.dma_start(out=rcp_sb[:], in_=rcp_d[:])
            identr = cp.tile([P, P], F32)
            nc.sync.dma_start(out=identr[:], in_=ident_d[:])
            identf = cp.tile([P, P], F32R)
            nc.vector.tensor_copy(out=identf[:], in_=identr[:])

            src_sb = arr.tile([P, S_tot], mybir.dt.int32)
            nc.sync.dma_start(out=src_sb[:], in_=src_d[:])

            def gps_dep(inst):
                return inst

            w0r = cp.tile([DIM + 1, FC], F32)
            nc.sync.dma_start(out=w0r[:], in_=w0_d[:])
            w0f = cp.tile([DIM + 1, FC], F32)
            nc.vector.tensor_copy(out=w0f[:], in_=w0r[:])
            w1r = cp.tile([FC + 1, FC], F32)
            nc.sync.dma_start(out=w1r[:], in_=w1_d[:])
            w1f = cp.tile([FC + 1, FC], F32R)
            nc.vector.tensor_copy(out=w1f[:], in_=w1r[:])
            w2r = cp.tile([FC + 1, OI], F32)
            nc.sync.dma_start(out=w2r[:], in_=w2_d[:])
            w2f = cp.tile([FC + 1, OI], F32R)
            nc.vector.tensor_copy(out=w2f[:], in_=w2r[:])

            lnr = cp.tile([1, 4 * FC], F32)
            nc.sync.dma_start(out=lnr[:], in_=ln_d[:])
            iotar = cp.tile([1, P], F32)
            nc.sync.dma_start(out=iotar[:], in_=iota_d[:])
            combo = cp.tile([1, 4 * FC + P], F32)
            nc.vector.tensor_copy(out=combo[:, :4 * FC], in_=lnr[:])
            nc.vector.tensor_copy(out=combo[:, 4 * FC:], in_=iotar[:])
            ones_row = cp.tile([1, P], F32)
            nc.vector.memset(ones_row[:], 1.0)
            eps_t = cp.tile([P, 1], F32)
            nc.vector.memset(eps_t[:], LN_EPS)

            with tc.tile_pool(name="pre_ps", bufs=1, space="PSUM") as pps:
                bc_ps = pps.tile([P, 4 * FC + P], F32, space="PSUM")
                nc.tensor.matmul(out=bc_ps[:], lhsT=ones_row[:], rhs=combo[:],
                                 start=True, stop=True)
                bc = cp.tile([P, 4 * FC + P], F32)
                nc.vector.tensor_copy(out=bc[:], in_=bc_ps[:])
            g0b = bc[:, 0:FC]
            b0b = bc[:, FC:2 * FC]
            g1b = bc[:, 2 * FC:3 * FC]
            b1b = bc[:, 3 * FC:4 * FC]
            iotab = bc[:, 4 * FC:]

            # persistent aug-transpose tiles (row FC is the bias/ones row),
            # ping-ponged to break the copy->matmul serialization chain
            h0nTs, h1nTs = [], []
            for _i in range(3):
                t0_ = cp.tile([FC + 1, P], F32R, tag=f"h0nT{_i}")
                nc.vector.tensor_copy(out=t0_[FC:FC + 1, :], in_=ones_row[:])
                h0nTs.append(t0_)
                t1_ = cp.tile([FC + 1, P], F32R, tag=f"h1nT{_i}")
                nc.vector.tensor_copy(out=t1_[FC:FC + 1, :], in_=ones_row[:])
                h1nTs.append(t1_)

            h0_all = arr.tile([P, FC * S_tot], F32R)
            h1_all = arr.tile([P, FC * S_tot], F32R)
            s_sum = arr.tile([P, S_tot], F32)
            s_sq = arr.tile([P, S_tot], F32)
            mu0 = arr.tile([P, S_tot], F32)
            rv0 = arr.tile([P, S_tot], F32)
            s1sum = arr.tile([P, S_tot], F32)
            s1sq = arr.tile([P, S_tot], F32)
            mu1 = arr.tile([P, S_tot], F32)
            rv1 = arr.tile([P, S_tot], F32)
            msq = arr.tile([P, S_tot], F32)

            # ---------------- phase 1: L0 + LN0 stats ----------------
            with tc.tile_pool(name="ps1", bufs=2, space="PSUM") as ps1:
                for (g0s, gn) in groups:
                    h0ps = ps1.tile([P, FC * 8], F32, space="PSUM", tag="h0g")
                    hsq = wk.tile([P, FC * 8], F32, tag="hsq")
                    xt = wk.tile([DIM + 1, P * 8], F32, tag="xt")
                    nc.sync.dma_start(out=xt[:, :P * gn],
                                      in_=xaug_d[:, P * g0s:P * (g0s + gn)])
                    for sl in range(gn):
                        s = g0s + sl
                        nc.tensor.matmul(
                            out=h0ps[:, FC * sl:FC * (sl + 1)],
                            lhsT=xt[:, P * sl:P * (sl + 1)],
                            rhs=w0f[:], start=True, stop=True)
                        nc.scalar.activation(
                            out=hsq[:, FC * sl:FC * (sl + 1)],
                            in_=h0ps[:, FC * sl:FC * (sl + 1)], func=AF.Square)
                    w = FC * gn
                    nc.vector.tensor_copy(
                        out=h0_all[:, FC * g0s:FC * (g0s + gn)], in_=h0ps[:, :w])
                    nc.vector.tensor_reduce(
                        out=s_sum[:, g0s:g0s + gn],
                        in_=h0ps[:, :w].rearrange("p (a i) -> p a i", i=FC),
                        axis=AX.X, op=OP.add)
                    nc.vector.tensor_reduce(
                        out=s_sq[:, g0s:g0s + gn],
                        in_=hsq[:, :w].rearrange("p (a i) -> p a i", i=FC),
                        axis=AX.X, op=OP.add)

            # ---------------- phase 2: LN small ops (batched) ----------------
            def ln_smalls(ssum, ssq, mu, rv):
                nc.vector.tensor_scalar(out=mu[:], in0=ssum[:], scalar1=1.0 / FC,
                                        scalar2=None, op0=OP.mult)
                nc.vector.tensor_scalar(out=ssq[:], in0=ssq[:], scalar1=1.0 / FC,
                                        scalar2=None, op0=OP.mult)
                nc.vector.tensor_tensor(out=msq[:], in0=mu[:], in1=mu[:],
                                        op=OP.mult)
                nc.vector.tensor_tensor(out=ssq[:], in0=ssq[:], in1=msq[:],
                                        op=OP.subtract)
                nc.scalar.activation(out=ssq[:], in_=ssq[:], func=AF.Sqrt,
                                     bias=eps_t[:, 0:1])
                nc.vector.reciprocal(out=rv[:], in_=ssq[:])

            ln_smalls(s_sum, s_sq, mu0, rv0)

            # ---------------- phase 3: normalize (in place) ----------------
            def normalize(h_all, mu, rv, gb, bb):
                for gi, (g0s, gn) in enumerate(groups):
                    use_gps = gi % 3 == 2
                    eng = nc.gpsimd if use_gps else nc.vector
                    hv = h_all[:, FC * g0s:FC * (g0s + gn)].rearrange(
                        "p (a i) -> p a i", i=FC)
                    mub = mu[:, g0s:g0s + gn].rearrange(
                        "p (a i) -> p a i", i=1).to_broadcast([P, gn, FC])
                    rvb = rv[:, g0s:g0s + gn].rearrange(
                        "p (a i) -> p a i", i=1).to_broadcast([P, gn, FC])
                    gbt = gb.rearrange("p (a i) -> p a i", a=1).to_broadcast(
                        [P, gn, FC])
                    bbt = bb.rearrange("p (a i) -> p a i", a=1).to_broadcast(
                        [P, gn, FC])
                    ops = [
                        eng.tensor_tensor(out=hv, in0=hv, in1=mub, op=OP.subtract),
                        eng.tensor_tensor(out=hv, in0=hv, in1=rvb, op=OP.mult),
                        eng.tensor_tensor(out=hv, in0=hv, in1=gbt, op=OP.mult),
                        eng.tensor_tensor(out=hv, in0=hv, in1=bbt, op=OP.add),
                    ]
                    relu = eng.tensor_scalar(
                        out=h_all[:, FC * g0s:FC * (g0s + gn)],
                        in0=h_all[:, FC * g0s:FC * (g0s + gn)],
                        scalar1=0.0, scalar2=None, op0=OP.max)
                    if use_gps:
                        for o in ops + [relu]:
                            gps_dep(o)

            normalize(h0_all, mu0, rv0, g0b, b0b)

            # ---------------- phase 4: transpose + L1 + LN1 stats ----------------
            with tc.tile_pool(name="ps4", bufs=2, space="PSUM") as ps4:
                for (g0s, gn) in groups:
                    h1ps = ps4.tile([P, FC * 8], F32, space="PSUM", tag="h1g")
                    hsq = wk.tile([P, FC * 8], F32, tag="hsq")
                    for sl in range(gn):
                        s = g0s + sl
                        t0ps = ps4.tile([FC, P], F32R, space="PSUM", tag="t0")
                        nc.tensor.transpose(
                            out=t0ps[:], in_=h0_all[:, FC * s:FC * (s + 1)],
                            identity=identf[:])
                        h0nT = h0nTs[s % 3]
                        nc.scalar.copy(out=h0nT[:FC, :], in_=t0ps[:])
                        nc.tensor.matmul(
                            out=h1ps[:, FC * sl:FC * (sl + 1)],
                            lhsT=h0nT[:], rhs=w1f[:], start=True, stop=True)
                        nc.scalar.activation(
                            out=hsq[:, FC * sl:FC * (sl + 1)],
                            in_=h1ps[:, FC * sl:FC * (sl + 1)], func=AF.Square)
                    w = FC * gn
                    nc.vector.tensor_copy(
                        out=h1_all[:, FC * g0s:FC * (g0s + gn)], in_=h1ps[:, :w])
                    nc.vector.tensor_reduce(
                        out=s1sum[:, g0s:g0s + gn],
                        in_=h1ps[:, :w].rearrange("p (a i) -> p a i", i=FC),
                        axis=AX.X, op=OP.add)
                    nc.vector.tensor_reduce(
                        out=s1sq[:, g0s:g0s + gn],
                        in_=hsq[:, :w].rearrange("p (a i) -> p a i", i=FC),
                        axis=AX.X, op=OP.add)

            # ---------------- phases 5+6: LN1 ----------------
            ln_smalls(s1sum, s1sq, mu1, rv1)
            normalize(h1_all, mu1, rv1, g1b, b1b)

            # ---------------- phase 7: L2 + tanh + mul + scatter ----------------
            with tc.tile_pool(name="ps7", bufs=1, space="PSUM") as psA, \
                 tc.tile_pool(name="ps7b", bufs=2, space="PSUM") as psB:
                for b in range(NBLK):
                    acc = psA.tile([P, OI], F32, space="PSUM", tag="acc")
                    for si in range(S):
                        s = b * S + si
                        oh = wk.tile([P, P], F32R, tag="oh")
                        nc.vector.tensor_tensor(
                            out=oh[:],
                            in0=tgt_sb[:, s:s + 1].to_broadcast([P, P]),
                            in1=iotab, op=OP.is_equal)
                        t1ps = psB.tile([FC, P], F32R, space="PSUM", tag="t1")
                        nc.tensor.transpose(
                            out=t1ps[:], in_=h1_all[:, FC * s:FC * (s + 1)],
                            identity=identf[:])
                        h1nT = h1nTs[s % 3]
                        nc.scalar.copy(out=h1nT[:FC, :], in_=t1ps[:])
                        zps = psB.tile([P, OI], F32, space="PSUM", tag="z")
                        nc.tensor.matmul(out=zps[:, 0:512], lhsT=h1nT[:],
                                         rhs=w2f[:, 0:512], start=True, stop=True)
                        nc.tensor.matmul(out=zps[:, 512:OI], lhsT=h1nT[:],
                                         rhs=w2f[:, 512:OI], start=True, stop=True)
                        t = wk.tile([P, OI], F32, tag="t")
                        nc.scalar.activation(out=t[:], in_=zps[:], func=AF.Tanh)
                        f = wk.tile([P, IN_C], F32, tag="f")
                        nc.gpsimd.indirect_dma_start(
                            out=f[:], out_offset=None, in_=feat_d[:],
                            in_offset=bass.IndirectOffsetOnAxis(
                                ap=src_sb[:, s:s + 1], axis=0))
                        prod = wk.tile([P, OI], F32R, tag="prod")
                        fb = f[:].rearrange("p (a i) -> p a i", a=1)
                        dg = DVE_OGROUPS
                        nc.vector.tensor_tensor(
                            out=_o_view(prod, 0, dg), in0=_o_view(t, 0, dg),
                            in1=fb.to_broadcast([P, dg, IN_C]), op=OP.mult)
                        gps_dep(nc.gpsimd.tensor_tensor(
                            out=_o_view(prod, dg, OUT_C), in0=_o_view(t, dg, OUT_C),
                            in1=fb.to_broadcast([P, OUT_C - dg, IN_C]), op=OP.mult))
                        nc.tensor.matmul(out=acc[:, 0:512], lhsT=oh[:],
                                         rhs=prod[:, 0:512], start=(si == 0),
                                         stop=(si == S - 1))
                        nc.tensor.matmul(out=acc[:, 512:OI], lhsT=oh[:],
                                         rhs=prod[:, 512:OI], start=(si == 0),
                                         stop=(si == S - 1))
                    ocv = wk.tile([P, OUT_C], F32, tag="ocv")
                    nc.vector.tensor_reduce(
                        out=ocv[:], in_=acc[:].rearrange("p (o i) -> p o i", i=IN_C),
                        axis=AX.X, op=OP.add)
                    osb = wk.tile([P, OUT_C], F32, tag="osb")
                    nc.vector.tensor_scalar(
                        out=osb[:], in0=ocv[:], scalar1=rcp_sb[:, b:b + 1],
                        scalar2=None, op0=OP.mult)
                    nc.sync.dma_start(out=out_d[P * b:P * (b + 1), :], in_=osb[:])

        for frm, to in gpsimd_deps:
            # add_dep_helper(A, B) == "A waits on B"
            add_dep_helper(to.ins, frm.ins, sync=False, reason="gpsimd library order")

    _patch_bass(nc)
    return nc


# ------------------------------------------------------------- host wrapper --
_cache = {}


def _get_kernel(S):
    if S not in _cache:
        _cache[S] = build_kernel(S)
    return _cache[S]


def kernel(features, hood_coords, source, target,
           W0, b0, g0, beta0, W1, b1, g1, beta1, W2, b2):
    features = np.asarray(features, dtype=np.float32)
    hood = np.asarray(hood_coords, dtype=np.float32)
    source = np.asarray(source).astype(np.int64)
    target = np.asarray(target).astype(np.int64)
    W0 = np.asarray(W0, dtype=np.float32)
    W1 = np.asarray(W1, dtype=np.float32)
    W2 = np.asarray(W2, dtype=np.float32)
    b0 = np.asarray(b0, dtype=np.float32)
    b1 = np.asarray(b1, dtype=np.float32)
    b2 = np.asarray(b2, dtype=np.float32)
    g0 = np.asarray(g0, dtype=np.float32)
    g1 = np.asarray(g1, dtype=np.float32)
    beta0 = np.asarray(beta0, dtype=np.float32)
    beta1 = np.asarray(beta1, dtype=np.float32)

    perm = np.argsort(target, kind="stable")
    tgt_s = target[perm]
    src_s = source[perm]
    hood_s = hood[perm]

    blk_starts = np.searchsorted(tgt_s, np.arange(0, N_CORES * V_CORE + 1, P))
    nseg = np.diff(blk_starts)
    S = max(1, int(np.ceil(nseg.max() / P)))
    S_tot = NBLK * S
    E_c = S_tot * P

    counts = np.bincount(target.astype(np.int64),
                         minlength=N_CORES * V_CORE).astype(np.float32)
    rcp_full = 1.0 / np.maximum(counts, 1.0)


    W0aug = np.vstack([W0 / RADIUS, b0[None, :]]).astype(np.float32)
    W1aug = np.vstack([W1, b1[None, :]]).astype(np.float32)
    W2aug = np.vstack([W2, b2[None, :]]).astype(np.float32)
    lnrow = np.concatenate([g0, beta0, g1, beta1])[None, :].astype(np.float32)
    iotarow = np.arange(P, dtype=np.float32)[None, :]
    ident = np.eye(P, dtype=np.float32)

    in_maps = []
    for k in range(N_CORES):
        xflat = np.zeros((E_c, DIM + 1), dtype=np.float32)
        xflat[:, DIM] = 1.0
        srcflat = np.zeros(E_c, dtype=np.int32)
        tgtflat = np.full(E_c, -1.0, dtype=np.float32)
        for b in range(NBLK):
            gi = k * NBLK + b
            e0, e1 = blk_starts[gi], blk_starts[gi + 1]
            n = e1 - e0
            if n == 0:
                continue
            p0 = b * S * P
            xflat[p0:p0 + n, :DIM] = hood_s[e0:e1]
            srcflat[p0:p0 + n] = src_s[e0:e1].astype(np.int32)
            tgtflat[p0:p0 + n] = (tgt_s[e0:e1] - (k * V_CORE + b * P)).astype(np.float32)
        rcp_k = rcp_full[k * V_CORE:(k + 1) * V_CORE].reshape(NBLK, P).T.copy()
        in_maps.append({
            "xaug": np.ascontiguousarray(xflat.T),
            "srcT": np.ascontiguousarray(srcflat.reshape(S_tot, P).T),
            "tgtloc": np.ascontiguousarray(tgtflat.reshape(S_tot, P).T),
            "feat": features,
            "W0aug": W0aug, "W1aug": W1aug, "W2aug": W2aug,
            "lnrow": lnrow, "iotarow": iotarow, "ident": ident,
            "rcp": np.ascontiguousarray(rcp_k),
        })

    nc = _get_kernel(S)
    res = run_bass_kernel_spmd(nc, in_maps, core_ids=list(range(N_CORES)))

    out = np.zeros((N_NODES, OUT_C), dtype=np.float32)
    for k in range(N_CORES):
        lo = k * V_CORE
        hi = min(lo + V_CORE, N_NODES)
        out[lo:hi] = res.results[k]["out"][:hi - lo]
    return out


# revision 15
# speedup vs baseline: 1.3434x; 1.0350x over previous
"""NeuralFieldCosmo GNN message-passing kernel for 8 Trainium2 NeuronCores.

Math (see reference):
  per-edge MLP on hood_coords: 3 -> 32 -> LN -> relu -> 32 -> LN -> relu
  -> 1024 -> tanh = per-edge weight matrix w[e, 32, 32]
  oc[e, o] = sum_i features[source[e], i] * w[e, o, i]
  out[v] = mean over edges with target[e] == v  (count-clamped)

Sharding: edges sorted by target; core k owns nodes [1280k, 1280k+1280).
Each core runs 10 node-blocks of 128 nodes; block edges padded to S
subtiles of 128 edges. Features are gathered per subtile by indirect DMA.
Scatter-mean is a one-hot matmul accumulated in PSUM per block; counts
are sharding metadata computed on host as reciprocals. Matmuls run in
fp32r (tf32-like). Output slices are concatenated on host.
"""
import numpy as np

import concourse.bass as bass
import concourse.mybir as mybir
import concourse.tile as tile
from concourse import library_config
from concourse.bass_utils import run_bass_kernel_spmd
from concourse.tile_rust import add_dep_helper

P = 128
IN_C = 32
OUT_C = 32
FC = 32
DIM = 3
OI = IN_C * OUT_C  # 1024
FPAD = 64               # feature rows padded to 64 floats (256B) for dma_gather
LN_EPS = 1e-5
RADIUS = 1.0
N_NODES = 10000
N_CORES = 8
V_CORE = 1280           # nodes per core (10 blocks of 128)
NBLK = V_CORE // P      # 10
DVE_OGROUPS = 26        # einsum mul split: o-groups on DVE, rest GPSIMD
F32 = mybir.dt.float32
F32R = mybir.dt.float32r
I16 = mybir.dt.int16
AX = mybir.AxisListType
OP = mybir.AluOpType
AF = mybir.ActivationFunctionType


# ---------------------------------------------------------------- legalize --
def _legalize_bir_json(bir: bytes) -> bytes:
    """This walrus build allows one sync wait per instruction; move excess
    waits onto NoOps inserted before the offending instruction."""
    import orjson
    j = orjson.loads(bir)
    ctr = 0
    for fn in j.get("functions", []):
        for blk in fn.get("blocks", []):
            out = []
            for ins in blk["instructions"]:
                si = ins.get("sync_info")
                waits = (si or {}).get("on_wait") or []
                if len(waits) > 1:
                    for w in waits[:-1]:
                        out.append({
                            "engine": ins.get("engine"), "ins": [],
                            "name": f"legalize-nop-{ctr}", "opcode": "NoOp",
                            "outs": [],
                            "sync_info": {"on_update": [], "on_wait": [w]},
                        })
                        ctr += 1
                    si["on_wait"] = [waits[-1]]
                out.append(ins)
            blk["instructions"] = out
    return orjson.dumps(j)


def _patch_bass(nc):
    orig = nc.to_json_bytes
    nc.to_json_bytes = lambda: _legalize_bir_json(orig())
    return nc


# ------------------------------------------------------------ device kernel --
def _o_view(ap, og0, og1):
    """[128, og*32 : og1*32] viewed as [128, og1-og0, 32]."""
    return ap[:, og0 * 32:og1 * 32].rearrange("p (o i) -> p o i", i=32)


def _r(ap):
    return ap.bitcast(F32R)


def build_kernel(S: int):
    S_tot = NBLK * S
    E_c = S_tot * P

    nc = bass.Bass()
    xaug_d = nc.dram_tensor("xaug", [DIM + 1, E_c], F32, kind="ExternalInput")
    src_d = nc.dram_tensor("srcT", [P, S_tot], mybir.dt.int32, kind="ExternalInput")
    tgt_d = nc.dram_tensor("tgtloc", [P, S_tot], F32, kind="ExternalInput")
    feat_d = nc.dram_tensor("feat", [N_NODES, IN_C], F32, kind="ExternalInput")
    w0_d = nc.dram_tensor("W0aug", [DIM + 1, FC], F32, kind="ExternalInput")
    w1_d = nc.dram_tensor("W1aug", [FC + 1, FC], F32, kind="ExternalInput")
    w2_d = nc.dram_tensor("W2aug", [FC + 1, OI], F32, kind="ExternalInput")
    ln_d = nc.dram_tensor("lnrow", [1, 4 * FC], F32, kind="ExternalInput")
    iota_d = nc.dram_tensor("iotarow", [1, P], F32, kind="ExternalInput")
    ident_d = nc.dram_tensor("ident", [P, P], F32, kind="ExternalInput")
    rcp_d = nc.dram_tensor("rcp", [P, NBLK], F32, kind="ExternalInput")
    out_d = nc.dram_tensor("out", [V_CORE, OUT_C], F32, kind="ExternalOutput")

    groups = []
    s0 = 0
    while s0 < S_tot:
        g = min(8, S_tot - s0)
        groups.append((s0, g))
        s0 += g

    gpsimd_deps = []   # (from_inst, to_inst) ordering for library correctness

    with tile.TileContext(nc) as tc:
        with (
            tc.tile_pool(name="const", bufs=1) as cp,
            tc.tile_pool(name="arrays", bufs=1) as arr,
            tc.tile_pool(name="work", bufs=3) as wk,
        ):
            # ---------------- preload constants ----------------
            tgt_sb = arr.tile([P, S_tot], F32)
            nc.sync.dma_start(out=tgt_sb[:], in_=tgt_d[:])
            rcp_sb = cp.tile([P, NBLK], F32)
            nc.sync.dma_start(out=rcp_sb[:], in_=rcp_d[:])
            identr = cp.tile([P, P], F32)
            nc.sync.dma_start(out=identr[:], in_=ident_d[:])
            identf = cp.tile([P, P], F32R)
            nc.vector.tensor_copy(out=identf[:], in_=identr[:])

            src_sb = arr.tile([P, S_tot], mybir.dt.int32)
            nc.sync.dma_start(out=src_sb[:], in_=src_d[:])

            def gps_dep(inst):
                return inst

            w0r = cp.tile([DIM + 1, FC], F32)
            nc.sync.dma_start(out=w0r[:], in_=w0_d[:])
            w0f = cp.tile([DIM + 1, FC], F32)
            nc.vector.tensor_copy(out=w0f[:], in_=w0r[:])
            w1r = cp.tile([FC + 1, FC], F32)
            nc.sync.dma_start(out=w1r[:], in_=w1_d[:])
            w1f = cp.tile([FC + 1, FC], F32R)
            nc.vector.tensor_copy(out=w1f[:], in_=w1r[:])
            w2r = cp.tile([FC + 1, OI], F32)
            nc.sync.dma_start(out=w2r[:], in_=w2_d[:])
            w2f = cp.tile([FC + 1, OI], F32R)
            nc.vector.tensor_copy(out=w2f[:], in_=w2r[:])

            lnr = cp.tile([1, 4 * FC], F32)
            nc.sync.dma_start(out=lnr[:], in_=ln_d[:])
            iotar = cp.tile([1, P], F32)
            nc.sync.dma_start(out=iotar[:], in_=iota_d[:])
            combo = cp.tile([1, 4 * FC + P], F32)
            nc.vector.tensor_copy(out=combo[:, :4 * FC], in_=lnr[:])
            nc.vector.tensor_copy(out=combo[:, 4 * FC:], in_=iotar[:])
            ones_row = cp.tile([1, P], F32)
            nc.vector.memset(ones_row[:], 1.0)
            eps_t = cp.tile([P, 1], F32)
            nc.vector.memset(eps_t[:], LN_EPS)

            with tc.tile_pool(name="pre_ps", bufs=1, space="PSUM") as pps:
                bc_ps = pps.tile([P, 4 * FC + P], F32, space="PSUM")
                nc.tensor.matmul(out=bc_ps[:], lhsT=ones_row[:], rhs=combo[:],
                                 start=True, stop=True)
                bc = cp.tile([P, 4 * FC + P], F32)
                nc.vector.tensor_copy(out=bc[:], in_=bc_ps[:])
            g0b = bc[:, 0:FC]
            b0b = bc[:, FC:2 * FC]
            g1b = bc[:, 2 * FC:3 * FC]
            b1b = bc[:, 3 * FC:4 * FC]
            iotab = bc[:, 4 * FC:]

            # persistent aug-transpose tiles (row FC is the bias/ones row),
            # ping-ponged to break the copy->matmul serialization chain
            h0nTs, h1nTs = [], []
            for _i in range(3):
                t0_ = cp.tile([FC + 1, P], F32R, tag=f"h0nT{_i}")
                nc.vector.tensor_copy(out=t0_[FC:FC + 1, :], in_=ones_row[:])
                h0nTs.append(t0_)
                t1_ = cp.tile([FC + 1, P], F32R, tag=f"h1nT{_i}")
                nc.vector.tensor_copy(out=t1_[FC:FC + 1, :], in_=ones_row[:])
                h1nTs.append(t1_)

            h0_all = arr.tile([P, FC * S_tot], F32R)
            h1_all = arr.tile([P, FC * S_tot], F32R)
            s_sum = arr.tile([P, S_tot], F32)
            s_sq = arr.tile([P, S_tot], F32)
            mu0 = arr.tile([P, S_tot], F32)
            rv0 = arr.tile([P, S_tot], F32)
            s1sum = arr.tile([P, S_tot], F32)
            s1sq = arr.tile([P, S_tot], F32)
            mu1 = arr.tile([P, S_tot], F32)
            rv1 = arr.tile([P, S_tot], F32)
            msq = arr.tile([P, S_tot], F32)

            # ---------------- phase 1: L0 + LN0 stats ----------------
            with tc.tile_pool(name="ps1", bufs=3, space="PSUM") as ps1:
                for (g0s, gn) in groups:
                    h0ps = ps1.tile([P, FC * 8], F32, space="PSUM", tag="h0g")
                    hsq = wk.tile([P, FC * 8], F32, tag="hsq")
                    xt = wk.tile([DIM + 1, P * 8], F32, tag="xt")
                    nc.sync.dma_start(out=xt[:, :P * gn],
                                      in_=xaug_d[:, P * g0s:P * (g0s + gn)])
                    for sl in range(gn):
                        s = g0s + sl
                        nc.tensor.matmul(
                            out=h0ps[:, FC * sl:FC * (sl + 1)],
                            lhsT=xt[:, P * sl:P * (sl + 1)],
                            rhs=w0f[:], start=True, stop=True)
                        nc.scalar.activation(
                            out=hsq[:, FC * sl:FC * (sl + 1)],
                            in_=h0ps[:, FC * sl:FC * (sl + 1)], func=AF.Square)
                    w = FC * gn
                    nc.vector.tensor_copy(
                        out=h0_all[:, FC * g0s:FC * (g0s + gn)], in_=h0ps[:, :w])
                    nc.vector.tensor_reduce(
                        out=s_sum[:, g0s:g0s + gn],
                        in_=h0ps[:, :w].rearrange("p (a i) -> p a i", i=FC),
                        axis=AX.X, op=OP.add)
                    nc.vector.tensor_reduce(
                        out=s_sq[:, g0s:g0s + gn],
                        in_=hsq[:, :w].rearrange("p (a i) -> p a i", i=FC),
                        axis=AX.X, op=OP.add)

            # ---------------- phase 2: LN small ops (batched) ----------------
            def ln_smalls(ssum, ssq, mu, rv):
                nc.vector.tensor_scalar(out=mu[:], in0=ssum[:], scalar1=1.0 / FC,
                                        scalar2=None, op0=OP.mult)
                nc.vector.tensor_scalar(out=ssq[:], in0=ssq[:], scalar1=1.0 / FC,
                                        scalar2=None, op0=OP.mult)
                nc.vector.tensor_tensor(out=msq[:], in0=mu[:], in1=mu[:],
                                        op=OP.mult)
                nc.vector.tensor_tensor(out=ssq[:], in0=ssq[:], in1=msq[:],
                                        op=OP.subtract)
                nc.scalar.activation(out=ssq[:], in_=ssq[:], func=AF.Sqrt,
                                     bias=eps_t[:, 0:1])
                nc.vector.reciprocal(out=rv[:], in_=ssq[:])

            ln_smalls(s_sum, s_sq, mu0, rv0)

            # ---------------- phase 3: normalize (in place) ----------------
            def normalize(h_all, mu, rv, gb, bb):
                for gi, (g0s, gn) in enumerate(groups):
                    use_gps = gi % 3 == 2
                    eng = nc.gpsimd if use_gps else nc.vector
                    hv = h_all[:, FC * g0s:FC * (g0s + gn)].rearrange(
                        "p (a i) -> p a i", i=FC)
                    mub = mu[:, g0s:g0s + gn].rearrange(
                        "p (a i) -> p a i", i=1).to_broadcast([P, gn, FC])
                    rvb = rv[:, g0s:g0s + gn].rearrange(
                        "p (a i) -> p a i", i=1).to_broadcast([P, gn, FC])
                    gbt = gb.rearrange("p (a i) -> p a i", a=1).to_broadcast(
                        [P, gn, FC])
                    bbt = bb.rearrange("p (a i) -> p a i", a=1).to_broadcast(
                        [P, gn, FC])
                    ops = [
                        eng.tensor_tensor(out=hv, in0=hv, in1=mub, op=OP.subtract),
                        eng.tensor_tensor(out=hv, in0=hv, in1=rvb, op=OP.mult),
                        eng.tensor_tensor(out=hv, in0=hv, in1=gbt, op=OP.mult),
                        eng.tensor_tensor(out=hv, in0=hv, in1=bbt, op=OP.add),
                    ]
                    relu = eng.tensor_scalar(
                        out=h_all[:, FC * g0s:FC * (g0s + gn)],
                        in0=h_all[:, FC * g0s:FC * (g0s + gn)],
                        scalar1=0.0, scalar2=None, op0=OP.max)
                    if use_gps:
                        for o in ops + [relu]:
                            gps_dep(o)

            normalize(h0_all, mu0, rv0, g0b, b0b)

            # ---------------- phase 4: transpose + L1 + LN1 stats ----------------
            with tc.tile_pool(name="ps4", bufs=3, space="PSUM") as ps4:
                for (g0s, gn) in groups:
                    h1ps = ps4.tile([P, FC * 8], F32, space="PSUM", tag="h1g")
                    hsq = wk.tile([P, FC * 8], F32, tag="hsq")
                    for sl in range(gn):
                        s = g0s + sl
                        t0ps = ps4.tile([FC, P], F32R, space="PSUM", tag="t0")
                        nc.tensor.transpose(
                            out=t0ps[:], in_=h0_all[:, FC * s:FC * (s + 1)],
                            identity=identf[:])
                        h0nT = h0nTs[s % 3]
                        nc.scalar.copy(out=h0nT[:FC, :], in_=t0ps[:])
                        nc.tensor.matmul(
                            out=h1ps[:, FC * sl:FC * (sl + 1)],
                            lhsT=h0nT[:], rhs=w1f[:], start=True, stop=True)
                        nc.scalar.activation(
                            out=hsq[:, FC * sl:FC * (sl + 1)],
                            in_=h1ps[:, FC * sl:FC * (sl + 1)], func=AF.Square)
                    w = FC * gn
                    nc.vector.tensor_copy(
                        out=h1_all[:, FC * g0s:FC * (g0s + gn)], in_=h1ps[:, :w])
                    nc.vector.tensor_reduce(
                        out=s1sum[:, g0s:g0s + gn],
                        in_=h1ps[:, :w].rearrange("p (a i) -> p a i", i=FC),
                        axis=AX.X, op=OP.add)
                    nc.vector.tensor_reduce(
                        out=s1sq[:, g0s:g0s + gn],
                        in_=hsq[:, :w].rearrange("p (a i) -> p a i", i=FC),
                        axis=AX.X, op=OP.add)

            # ---------------- phases 5+6: LN1 ----------------
            ln_smalls(s1sum, s1sq, mu1, rv1)
            normalize(h1_all, mu1, rv1, g1b, b1b)

            # ---------------- phase 7: L2 + tanh + mul + scatter ----------------
            with tc.tile_pool(name="ps7", bufs=1, space="PSUM") as psA, \
                 tc.tile_pool(name="ps7b", bufs=2, space="PSUM") as psB:
                for b in range(NBLK):
                    acc = psA.tile([P, OI], F32, space="PSUM", tag="acc")
                    for si in range(S):
                        s = b * S + si
                        oh = wk.tile([P, P], F32R, tag="oh")
                        nc.vector.tensor_tensor(
                            out=oh[:],
                            in0=tgt_sb[:, s:s + 1].to_broadcast([P, P]),
                            in1=iotab, op=OP.is_equal)
                        t1ps = psB.tile([FC, P], F32R, space="PSUM", tag="t1")
                        nc.tensor.transpose(
                            out=t1ps[:], in_=h1_all[:, FC * s:FC * (s + 1)],
                            identity=identf[:])
                        h1nT = h1nTs[s % 3]
                        nc.scalar.copy(out=h1nT[:FC, :], in_=t1ps[:])
                        zps = psB.tile([P, OI], F32, space="PSUM", tag="z")
                        nc.tensor.matmul(out=zps[:, 0:512], lhsT=h1nT[:],
                                         rhs=w2f[:, 0:512], start=True, stop=True)
                        nc.tensor.matmul(out=zps[:, 512:OI], lhsT=h1nT[:],
                                         rhs=w2f[:, 512:OI], start=True, stop=True)
                        t = wk.tile([P, OI], F32, tag="t")
                        nc.scalar.activation(out=t[:], in_=zps[:], func=AF.Tanh)
                        f = wk.tile([P, IN_C], F32, tag="f")
                        nc.gpsimd.indirect_dma_start(
                            out=f[:], out_offset=None, in_=feat_d[:],
                            in_offset=bass.IndirectOffsetOnAxis(
                                ap=src_sb[:, s:s + 1], axis=0))
                        prod = wk.tile([P, OI], F32R, tag="prod")
                        fb = f[:].rearrange("p (a i) -> p a i", a=1)
                        dg = DVE_OGROUPS
                        nc.vector.tensor_tensor(
                            out=_o_view(prod, 0, dg), in0=_o_view(t, 0, dg),
                            in1=fb.to_broadcast([P, dg, IN_C]), op=OP.mult)
                        gps_dep(nc.gpsimd.tensor_tensor(
                            out=_o_view(prod, dg, OUT_C), in0=_o_view(t, dg, OUT_C),
                            in1=fb.to_broadcast([P, OUT_C - dg, IN_C]), op=OP.mult))
                        nc.tensor.matmul(out=acc[:, 0:512], lhsT=oh[:],
                                         rhs=prod[:, 0:512], start=(si == 0),
                                         stop=(si == S - 1))
                        nc.tensor.matmul(out=acc[:, 512:OI], lhsT=oh[:],
                                         rhs=prod[:, 512:OI], start=(si == 0),
                                         stop=(si == S - 1))
                    ocv = wk.tile([P, OUT_C], F32, tag="ocv")
                    nc.vector.tensor_reduce(
                        out=ocv[:], in_=acc[:].rearrange("p (o i) -> p o i", i=IN_C),
                        axis=AX.X, op=OP.add)
                    osb = wk.tile([P, OUT_C], F32, tag="osb")
                    nc.vector.tensor_scalar(
                        out=osb[:], in0=ocv[:], scalar1=rcp_sb[:, b:b + 1],
                        scalar2=None, op0=OP.mult)
                    nc.sync.dma_start(out=out_d[P * b:P * (b + 1), :], in_=osb[:])

        for frm, to in gpsimd_deps:
            # add_dep_helper(A, B) == "A waits on B"
            add_dep_helper(to.ins, frm.ins, sync=False, reason="gpsimd library order")

    _patch_bass(nc)
    return nc


# ------------------------------------------------------------- host wrapper --
_cache = {}


def _get_kernel(S):
    if S not in _cache:
        _cache[S] = build_kernel(S)
    return _cache[S]


def kernel(features, hood_coords, source, target,
           W0, b0, g0, beta0, W1, b1, g1, beta1, W2, b2):
    features = np.asarray(features, dtype=np.float32)
    hood = np.asarray(hood_coords, dtype=np.float32)
    source = np.asarray(source).astype(np.int64)
    target = np.asarray(target).astype(np.int64)
    W0 = np.asarray(W0, dtype=np.float32)
    W1 = np.asarray(W1, dtype=np.float32)
    W2 = np.asarray(W2, dtype=np.float32)
    b0 = np.asarray(b0, dtype=np.float32)
    b1 = np.asarray(b1, dtype=np.float32)
    b2 = np.asarray(b2, dtype=np.float32)
    g0 = np.asarray(g0, dtype=np.float32)
    g1 = np.asarray(g1, dtype=np.float32)
    beta0 = np.asarray(beta0, dtype=np.float32)
    beta1 = np.asarray(beta1, dtype=np.float32)

    perm = np.argsort(target, kind="stable")
    tgt_s = target[perm]
    src_s = source[perm]
    hood_s = hood[perm]

    blk_starts = np.searchsorted(tgt_s, np.arange(0, N_CORES * V_CORE + 1, P))
    nseg = np.diff(blk_starts)
    S = max(1, int(np.ceil(nseg.max() / P)))
    S_tot = NBLK * S
    E_c = S_tot * P

    counts = np.bincount(target.astype(np.int64),
                         minlength=N_CORES * V_CORE).astype(np.float32)
    rcp_full = 1.0 / np.maximum(counts, 1.0)


    W0aug = np.vstack([W0 / RADIUS, b0[None, :]]).astype(np.float32)
    W1aug = np.vstack([W1, b1[None, :]]).astype(np.float32)
    W2aug = np.vstack([W2, b2[None, :]]).astype(np.float32)
    lnrow = np.concatenate([g0, beta0, g1, beta1])[None, :].astype(np.float32)
    iotarow = np.arange(P, dtype=np.float32)[None, :]
    ident = np.eye(P, dtype=np.float32)

    in_maps = []
    for k in range(N_CORES):
        xflat = np.zeros((E_c, DIM + 1), dtype=np.float32)
        xflat[:, DIM] = 1.0
        srcflat = np.zeros(E_c, dtype=np.int32)
        tgtflat = np.full(E_c, -1.0, dtype=np.float32)
        for b in range(NBLK):
            gi = k * NBLK + b
            e0, e1 = blk_starts[gi], blk_starts[gi + 1]
            n = e1 - e0
            if n == 0:
                continue
            p0 = b * S * P
            xflat[p0:p0 + n, :DIM] = hood_s[e0:e1]
            srcflat[p0:p0 + n] = src_s[e0:e1].astype(np.int32)
            tgtflat[p0:p0 + n] = (tgt_s[e0:e1] - (k * V_CORE + b * P)).astype(np.float32)
        rcp_k = rcp_full[k * V_CORE:(k + 1) * V_CORE].reshape(NBLK, P).T.copy()
        in_maps.append({
            "xaug": np.ascontiguousarray(xflat.T),
            "srcT": np.ascontiguousarray(srcflat.reshape(S_tot, P).T),
            "tgtloc": np.ascontiguousarray(tgtflat.reshape(S_tot, P).T),
            "feat": features,
            "W0aug": W0aug, "W1aug": W1aug, "W2aug": W2aug,
            "lnrow": lnrow, "iotarow": iotarow, "ident": ident,
            "rcp": np.ascontiguousarray(rcp_k),
        })

    nc = _get_kernel(S)
    res = run_bass_kernel_spmd(nc, in_maps, core_ids=list(range(N_CORES)))

    out = np.zeros((N_NODES, OUT_C), dtype=np.float32)
    for k in range(N_CORES):
        lo = k * V_CORE
        hi = min(lo + V_CORE, N_NODES)
        out[lo:hi] = res.results[k]["out"][:hi - lo]
    return out


# revision 16
# speedup vs baseline: 1.3863x; 1.0320x over previous
"""NeuralFieldCosmo GNN message-passing kernel for 8 Trainium2 NeuronCores.

Math (see reference):
  per-edge MLP on hood_coords: 3 -> 32 -> LN -> relu -> 32 -> LN -> relu
  -> 1024 -> tanh = per-edge weight matrix w[e, 32, 32]
  oc[e, o] = sum_i features[source[e], i] * w[e, o, i]
  out[v] = mean over edges with target[e] == v  (count-clamped)

Sharding: edges sorted by target; core k owns nodes [1280k, 1280k+1280).
Each core runs 10 node-blocks of 128 nodes; block edges padded to S
subtiles of 128 edges. Features are gathered per subtile by indirect DMA.
Scatter-mean is a one-hot matmul accumulated in PSUM per block; counts
are sharding metadata computed on host as reciprocals. Matmuls run in
fp32r (tf32-like). Output slices are concatenated on host.
"""
import numpy as np

import concourse.bass as bass
import concourse.mybir as mybir
import concourse.tile as tile
from concourse import library_config
from concourse.bass_utils import run_bass_kernel_spmd
from concourse.tile_rust import add_dep_helper

P = 128
IN_C = 32
OUT_C = 32
FC = 32
DIM = 3
OI = IN_C * OUT_C  # 1024
FPAD = 64               # feature rows padded to 64 floats (256B) for dma_gather
LN_EPS = 1e-5
RADIUS = 1.0
N_NODES = 10000
N_CORES = 8
V_CORE = 1280           # nodes per core (10 blocks of 128)
NBLK = V_CORE // P      # 10
DVE_OGROUPS = 26        # einsum mul split: o-groups on DVE, rest GPSIMD
F32 = mybir.dt.float32
F32R = mybir.dt.float32r
I16 = mybir.dt.int16
AX = mybir.AxisListType
OP = mybir.AluOpType
AF = mybir.ActivationFunctionType


# ---------------------------------------------------------------- legalize --
def _legalize_bir_json(bir: bytes) -> bytes:
    """This walrus build allows one sync wait per instruction; move excess
    waits onto NoOps inserted before the offending instruction."""
    import orjson
    j = orjson.loads(bir)
    ctr = 0
    for fn in j.get("functions", []):
        for blk in fn.get("blocks", []):
            out = []
            for ins in blk["instructions"]:
                si = ins.get("sync_info")
                waits = (si or {}).get("on_wait") or []
                if len(waits) > 1:
                    for w in waits[:-1]:
                        out.append({
                            "engine": ins.get("engine"), "ins": [],
                            "name": f"legalize-nop-{ctr}", "opcode": "NoOp",
                            "outs": [],
                            "sync_info": {"on_update": [], "on_wait": [w]},
                        })
                        ctr += 1
                    si["on_wait"] = [waits[-1]]
                out.append(ins)
            blk["instructions"] = out
    return orjson.dumps(j)


def _patch_bass(nc):
    orig = nc.to_json_bytes
    nc.to_json_bytes = lambda: _legalize_bir_json(orig())
    return nc


# ------------------------------------------------------------ device kernel --
def _o_view(ap, og0, og1):
    """[128, og*32 : og1*32] viewed as [128, og1-og0, 32]."""
    return ap[:, og0 * 32:og1 * 32].rearrange("p (o i) -> p o i", i=32)


def _r(ap):
    return ap.bitcast(F32R)


def build_kernel(S: int):
    S_tot = NBLK * S
    E_c = S_tot * P

    nc = bass.Bass()
    xaug_d = nc.dram_tensor("xaug", [DIM + 1, E_c], F32, kind="ExternalInput")
    src_d = nc.dram_tensor("srcT", [P, S_tot], mybir.dt.int32, kind="ExternalInput")
    tgt_d = nc.dram_tensor("tgtloc", [P, S_tot], F32, kind="ExternalInput")
    feat_d = nc.dram_tensor("feat", [N_NODES, IN_C], F32, kind="ExternalInput")
    w0_d = nc.dram_tensor("W0aug", [DIM + 1, FC], F32, kind="ExternalInput")
    w1_d = nc.dram_tensor("W1aug", [FC + 1, FC], F32, kind="ExternalInput")
    w2_d = nc.dram_tensor("W2aug", [FC + 1, OI], F32, kind="ExternalInput")
    ln_d = nc.dram_tensor("lnrow", [1, 4 * FC], F32, kind="ExternalInput")
    lncol_d = nc.dram_tensor("lncol", [FC, 4], F32, kind="ExternalInput")
    iota_d = nc.dram_tensor("iotarow", [1, P], F32, kind="ExternalInput")
    ident_d = nc.dram_tensor("ident", [P, P], F32, kind="ExternalInput")
    rcp_d = nc.dram_tensor("rcp", [P, NBLK], F32, kind="ExternalInput")
    out_d = nc.dram_tensor("out", [V_CORE, OUT_C], F32, kind="ExternalOutput")

    groups = []
    s0 = 0
    while s0 < S_tot:
        g = min(8, S_tot - s0)
        groups.append((s0, g))
        s0 += g

    gpsimd_deps = []   # (from_inst, to_inst) ordering for library correctness

    with tile.TileContext(nc) as tc:
        with (
            tc.tile_pool(name="const", bufs=1) as cp,
            tc.tile_pool(name="arrays", bufs=1) as arr,
            tc.tile_pool(name="work", bufs=3) as wk,
        ):
            # ---------------- preload constants ----------------
            tgt_sb = arr.tile([P, S_tot], F32)
            nc.sync.dma_start(out=tgt_sb[:], in_=tgt_d[:])
            rcp_sb = cp.tile([P, NBLK], F32)
            nc.sync.dma_start(out=rcp_sb[:], in_=rcp_d[:])
            identr = cp.tile([P, P], F32)
            nc.sync.dma_start(out=identr[:], in_=ident_d[:])
            identf = cp.tile([P, P], F32R)
            nc.vector.tensor_copy(out=identf[:], in_=identr[:])

            src_sb = arr.tile([P, S_tot], mybir.dt.int32)
            nc.sync.dma_start(out=src_sb[:], in_=src_d[:])

            def gps_dep(inst):
                return inst

            w0r = cp.tile([DIM + 1, FC], F32)
            nc.sync.dma_start(out=w0r[:], in_=w0_d[:])
            w0f = cp.tile([DIM + 1, FC], F32)
            nc.vector.tensor_copy(out=w0f[:], in_=w0r[:])
            w1r = cp.tile([FC + 1, FC], F32)
            nc.sync.dma_start(out=w1r[:], in_=w1_d[:])
            w1f = cp.tile([FC + 1, FC], F32R)
            nc.vector.tensor_copy(out=w1f[:], in_=w1r[:])
            w2r = cp.tile([FC + 1, OI], F32)
            nc.sync.dma_start(out=w2r[:], in_=w2_d[:])
            w2f = cp.tile([FC + 1, OI], F32R)
            nc.vector.tensor_copy(out=w2f[:], in_=w2r[:])

            lnr = cp.tile([1, 4 * FC], F32)
            nc.sync.dma_start(out=lnr[:], in_=ln_d[:])
            lncol = cp.tile([FC, 4], F32)
            nc.sync.dma_start(out=lncol[:], in_=lncol_d[:])
            iotar = cp.tile([1, P], F32)
            nc.sync.dma_start(out=iotar[:], in_=iota_d[:])
            combo = cp.tile([1, 4 * FC + P], F32)
            nc.vector.tensor_copy(out=combo[:, :4 * FC], in_=lnr[:])
            nc.vector.tensor_copy(out=combo[:, 4 * FC:], in_=iotar[:])
            ones_row = cp.tile([1, P], F32)
            nc.vector.memset(ones_row[:], 1.0)
            eps_t = cp.tile([P, 1], F32)
            nc.vector.memset(eps_t[:], LN_EPS)

            with tc.tile_pool(name="pre_ps", bufs=1, space="PSUM") as pps:
                bc_ps = pps.tile([P, 4 * FC + P], F32, space="PSUM")
                nc.tensor.matmul(out=bc_ps[:], lhsT=ones_row[:], rhs=combo[:],
                                 start=True, stop=True)
                bc = cp.tile([P, 4 * FC + P], F32)
                nc.vector.tensor_copy(out=bc[:], in_=bc_ps[:])
            g0b = bc[:, 0:FC]
            b0b = bc[:, FC:2 * FC]
            g1b = bc[:, 2 * FC:3 * FC]
            b1b = bc[:, 3 * FC:4 * FC]
            iotab = bc[:, 4 * FC:]

            # persistent aug-transpose tiles (row FC is the bias/ones row),
            # ping-ponged to break the copy->matmul serialization chain
            h0nTs, h1nTs = [], []
            for _i in range(3):
                t0_ = cp.tile([FC + 1, P], F32R, tag=f"h0nT{_i}")
                nc.vector.tensor_copy(out=t0_[FC:FC + 1, :], in_=ones_row[:])
                h0nTs.append(t0_)
                t1_ = cp.tile([FC + 1, P], F32R, tag=f"h1nT{_i}")
                nc.vector.tensor_copy(out=t1_[FC:FC + 1, :], in_=ones_row[:])
                h1nTs.append(t1_)

            h0_all = arr.tile([P, FC * S_tot], F32R)
            h1_all = arr.tile([P, FC * S_tot], F32R)
            s_sum = arr.tile([P, S_tot], F32)
            s_sq = arr.tile([P, S_tot], F32)
            mu0 = arr.tile([P, S_tot], F32)
            rv0 = arr.tile([P, S_tot], F32)
            s1sum = arr.tile([P, S_tot], F32)
            s1sq = arr.tile([P, S_tot], F32)
            mu1 = arr.tile([P, S_tot], F32)
            rv1 = arr.tile([P, S_tot], F32)
            msq = arr.tile([P, S_tot], F32)

            # ---------------- phase 1: L0 + LN0 stats ----------------
            with tc.tile_pool(name="ps1", bufs=3, space="PSUM") as ps1:
                for (g0s, gn) in groups:
                    h0ps = ps1.tile([P, FC * 8], F32, space="PSUM", tag="h0g")
                    hsq = wk.tile([P, FC * 8], F32, tag="hsq")
                    xt = wk.tile([DIM + 1, P * 8], F32, tag="xt")
                    nc.sync.dma_start(out=xt[:, :P * gn],
                                      in_=xaug_d[:, P * g0s:P * (g0s + gn)])
                    for sl in range(gn):
                        s = g0s + sl
                        nc.tensor.matmul(
                            out=h0ps[:, FC * sl:FC * (sl + 1)],
                            lhsT=xt[:, P * sl:P * (sl + 1)],
                            rhs=w0f[:], start=True, stop=True)
                        nc.scalar.activation(
                            out=hsq[:, FC * sl:FC * (sl + 1)],
                            in_=h0ps[:, FC * sl:FC * (sl + 1)], func=AF.Square)
                    w = FC * gn
                    nc.vector.tensor_copy(
                        out=h0_all[:, FC * g0s:FC * (g0s + gn)], in_=h0ps[:, :w])
                    nc.vector.tensor_reduce(
                        out=s_sum[:, g0s:g0s + gn],
                        in_=h0ps[:, :w].rearrange("p (a i) -> p a i", i=FC),
                        axis=AX.X, op=OP.add)
                    nc.vector.tensor_reduce(
                        out=s_sq[:, g0s:g0s + gn],
                        in_=hsq[:, :w].rearrange("p (a i) -> p a i", i=FC),
                        axis=AX.X, op=OP.add)

            # ---------------- phase 2: LN small ops (batched) ----------------
            def ln_smalls(ssum, ssq, mu, rv):
                nc.vector.tensor_scalar(out=mu[:], in0=ssum[:], scalar1=1.0 / FC,
                                        scalar2=None, op0=OP.mult)
                nc.vector.tensor_scalar(out=ssq[:], in0=ssq[:], scalar1=1.0 / FC,
                                        scalar2=None, op0=OP.mult)
                nc.vector.tensor_tensor(out=msq[:], in0=mu[:], in1=mu[:],
                                        op=OP.mult)
                nc.vector.tensor_tensor(out=ssq[:], in0=ssq[:], in1=msq[:],
                                        op=OP.subtract)
                nc.scalar.activation(out=ssq[:], in_=ssq[:], func=AF.Sqrt,
                                     bias=eps_t[:, 0:1])
                nc.vector.reciprocal(out=rv[:], in_=ssq[:])

            ln_smalls(s_sum, s_sq, mu0, rv0)

            # ---------------- phase 3: normalize (in place) ----------------
            def normalize(h_all, mu, rv, gb, bb):
                for gi, (g0s, gn) in enumerate(groups):
                    use_gps = gi % 3 == 2
                    eng = nc.gpsimd if use_gps else nc.vector
                    hv = h_all[:, FC * g0s:FC * (g0s + gn)].rearrange(
                        "p (a i) -> p a i", i=FC)
                    mub = mu[:, g0s:g0s + gn].rearrange(
                        "p (a i) -> p a i", i=1).to_broadcast([P, gn, FC])
                    rvb = rv[:, g0s:g0s + gn].rearrange(
                        "p (a i) -> p a i", i=1).to_broadcast([P, gn, FC])
                    gbt = gb.rearrange("p (a i) -> p a i", a=1).to_broadcast(
                        [P, gn, FC])
                    bbt = bb.rearrange("p (a i) -> p a i", a=1).to_broadcast(
                        [P, gn, FC])
                    ops = [
                        eng.tensor_tensor(out=hv, in0=hv, in1=mub, op=OP.subtract),
                        eng.tensor_tensor(out=hv, in0=hv, in1=rvb, op=OP.mult),
                    ]
                    if use_gps:
                        for o in ops:
                            gps_dep(o)

            normalize(h0_all, mu0, rv0, g0b, b0b)

            # ---------------- phase 4: transpose + L1 + LN1 stats ----------------
            with tc.tile_pool(name="ps4", bufs=3, space="PSUM") as ps4:
                for (g0s, gn) in groups:
                    h1ps = ps4.tile([P, FC * 8], F32, space="PSUM", tag="h1g")
                    hsq = wk.tile([P, FC * 8], F32, tag="hsq")
                    for sl in range(gn):
                        s = g0s + sl
                        t0ps = ps4.tile([FC, P], F32R, space="PSUM", tag="t0")
                        nc.tensor.transpose(
                            out=t0ps[:], in_=h0_all[:, FC * s:FC * (s + 1)],
                            identity=identf[:])
                        h0nT = h0nTs[s % 3]
                        nc.scalar.activation(
                            out=h0nT[:FC, :], in_=t0ps[:], func=AF.Relu,
                            scale=lncol[:, 0:1], bias=lncol[:, 1:2])
                        nc.tensor.matmul(
                            out=h1ps[:, FC * sl:FC * (sl + 1)],
                            lhsT=h0nT[:], rhs=w1f[:], start=True, stop=True)
                        nc.scalar.activation(
                            out=hsq[:, FC * sl:FC * (sl + 1)],
                            in_=h1ps[:, FC * sl:FC * (sl + 1)], func=AF.Square)
                    w = FC * gn
                    nc.vector.tensor_copy(
                        out=h1_all[:, FC * g0s:FC * (g0s + gn)], in_=h1ps[:, :w])
                    nc.vector.tensor_reduce(
                        out=s1sum[:, g0s:g0s + gn],
                        in_=h1ps[:, :w].rearrange("p (a i) -> p a i", i=FC),
                        axis=AX.X, op=OP.add)
                    nc.vector.tensor_reduce(
                        out=s1sq[:, g0s:g0s + gn],
                        in_=hsq[:, :w].rearrange("p (a i) -> p a i", i=FC),
                        axis=AX.X, op=OP.add)

            # ---------------- phases 5+6: LN1 ----------------
            ln_smalls(s1sum, s1sq, mu1, rv1)
            normalize(h1_all, mu1, rv1, g1b, b1b)

            # ---------------- phase 7: L2 + tanh + mul + scatter ----------------
            with tc.tile_pool(name="ps7", bufs=1, space="PSUM") as psA, \
                 tc.tile_pool(name="ps7b", bufs=2, space="PSUM") as psB:
                for b in range(NBLK):
                    acc = psA.tile([P, OI], F32, space="PSUM", tag="acc")
                    for si in range(S):
                        s = b * S + si
                        oh = wk.tile([P, P], F32R, tag="oh")
                        nc.vector.tensor_tensor(
                            out=oh[:],
                            in0=tgt_sb[:, s:s + 1].to_broadcast([P, P]),
                            in1=iotab, op=OP.is_equal)
                        t1ps = psB.tile([FC, P], F32R, space="PSUM", tag="t1")
                        nc.tensor.transpose(
                            out=t1ps[:], in_=h1_all[:, FC * s:FC * (s + 1)],
                            identity=identf[:])
                        h1nT = h1nTs[s % 3]
                        nc.scalar.activation(
                            out=h1nT[:FC, :], in_=t1ps[:], func=AF.Relu,
                            scale=lncol[:, 2:3], bias=lncol[:, 3:4])
                        zps = psB.tile([P, OI], F32, space="PSUM", tag="z")
                        nc.tensor.matmul(out=zps[:, 0:512], lhsT=h1nT[:],
                                         rhs=w2f[:, 0:512], start=True, stop=True)
                        nc.tensor.matmul(out=zps[:, 512:OI], lhsT=h1nT[:],
                                         rhs=w2f[:, 512:OI], start=True, stop=True)
                        t = wk.tile([P, OI], F32, tag="t")
                        nc.scalar.activation(out=t[:], in_=zps[:], func=AF.Tanh)
                        f = wk.tile([P, IN_C], F32, tag="f")
                        nc.gpsimd.indirect_dma_start(
                            out=f[:], out_offset=None, in_=feat_d[:],
                            in_offset=bass.IndirectOffsetOnAxis(
                                ap=src_sb[:, s:s + 1], axis=0))
                        prod = wk.tile([P, OI], F32R, tag="prod")
                        fb = f[:].rearrange("p (a i) -> p a i", a=1)
                        dg = DVE_OGROUPS
                        nc.vector.tensor_tensor(
                            out=_o_view(prod, 0, dg), in0=_o_view(t, 0, dg),
                            in1=fb.to_broadcast([P, dg, IN_C]), op=OP.mult)
                        gps_dep(nc.gpsimd.tensor_tensor(
                            out=_o_view(prod, dg, OUT_C), in0=_o_view(t, dg, OUT_C),
                            in1=fb.to_broadcast([P, OUT_C - dg, IN_C]), op=OP.mult))
                        nc.tensor.matmul(out=acc[:, 0:512], lhsT=oh[:],
                                         rhs=prod[:, 0:512], start=(si == 0),
                                         stop=(si == S - 1))
                        nc.tensor.matmul(out=acc[:, 512:OI], lhsT=oh[:],
                                         rhs=prod[:, 512:OI], start=(si == 0),
                                         stop=(si == S - 1))
                    ocv = wk.tile([P, OUT_C], F32, tag="ocv")
                    nc.vector.tensor_reduce(
                        out=ocv[:], in_=acc[:].rearrange("p (o i) -> p o i", i=IN_C),
                        axis=AX.X, op=OP.add)
                    osb = wk.tile([P, OUT_C], F32, tag="osb")
                    nc.vector.tensor_scalar(
                        out=osb[:], in0=ocv[:], scalar1=rcp_sb[:, b:b + 1],
                        scalar2=None, op0=OP.mult)
                    nc.sync.dma_start(out=out_d[P * b:P * (b + 1), :], in_=osb[:])

        for frm, to in gpsimd_deps:
            # add_dep_helper(A, B) == "A waits on B"
            add_dep_helper(to.ins, frm.ins, sync=False, reason="gpsimd library order")

    _patch_bass(nc)
    return nc


# ------------------------------------------------------------- host wrapper --
_cache = {}


def _get_kernel(S):
    if S not in _cache:
        _cache[S] = build_kernel(S)
    return _cache[S]


def kernel(features, hood_coords, source, target,
           W0, b0, g0, beta0, W1, b1, g1, beta1, W2, b2):
    features = np.asarray(features, dtype=np.float32)
    hood = np.asarray(hood_coords, dtype=np.float32)
    source = np.asarray(source).astype(np.int64)
    target = np.asarray(target).astype(np.int64)
    W0 = np.asarray(W0, dtype=np.float32)
    W1 = np.asarray(W1, dtype=np.float32)
    W2 = np.asarray(W2, dtype=np.float32)
    b0 = np.asarray(b0, dtype=np.float32)
    b1 = np.asarray(b1, dtype=np.float32)
    b2 = np.asarray(b2, dtype=np.float32)
    g0 = np.asarray(g0, dtype=np.float32)
    g1 = np.asarray(g1, dtype=np.float32)
    beta0 = np.asarray(beta0, dtype=np.float32)
    beta1 = np.asarray(beta1, dtype=np.float32)

    perm = np.argsort(target, kind="stable")
    tgt_s = target[perm]
    src_s = source[perm]
    hood_s = hood[perm]

    blk_starts = np.searchsorted(tgt_s, np.arange(0, N_CORES * V_CORE + 1, P))
    nseg = np.diff(blk_starts)
    S = max(1, int(np.ceil(nseg.max() / P)))
    S_tot = NBLK * S
    E_c = S_tot * P

    counts = np.bincount(target.astype(np.int64),
                         minlength=N_CORES * V_CORE).astype(np.float32)
    rcp_full = 1.0 / np.maximum(counts, 1.0)


    W0aug = np.vstack([W0 / RADIUS, b0[None, :]]).astype(np.float32)
    W1aug = np.vstack([W1, b1[None, :]]).astype(np.float32)
    W2aug = np.vstack([W2, b2[None, :]]).astype(np.float32)
    lnrow = np.concatenate([g0, beta0, g1, beta1])[None, :].astype(np.float32)
    lncol = np.stack([g0, beta0, g1, beta1], axis=1).astype(np.float32)
    iotarow = np.arange(P, dtype=np.float32)[None, :]
    ident = np.eye(P, dtype=np.float32)

    in_maps = []
    for k in range(N_CORES):
        xflat = np.zeros((E_c, DIM + 1), dtype=np.float32)
        xflat[:, DIM] = 1.0
        srcflat = np.zeros(E_c, dtype=np.int32)
        tgtflat = np.full(E_c, -1.0, dtype=np.float32)
        for b in range(NBLK):
            gi = k * NBLK + b
            e0, e1 = blk_starts[gi], blk_starts[gi + 1]
            n = e1 - e0
            if n == 0:
                continue
            p0 = b * S * P
            xflat[p0:p0 + n, :DIM] = hood_s[e0:e1]
            srcflat[p0:p0 + n] = src_s[e0:e1].astype(np.int32)
            tgtflat[p0:p0 + n] = (tgt_s[e0:e1] - (k * V_CORE + b * P)).astype(np.float32)
        rcp_k = rcp_full[k * V_CORE:(k + 1) * V_CORE].reshape(NBLK, P).T.copy()
        in_maps.append({
            "xaug": np.ascontiguousarray(xflat.T),
            "srcT": np.ascontiguousarray(srcflat.reshape(S_tot, P).T),
            "tgtloc": np.ascontiguousarray(tgtflat.reshape(S_tot, P).T),
            "feat": features,
            "W0aug": W0aug, "W1aug": W1aug, "W2aug": W2aug,
            "lnrow": lnrow, "lncol": lncol, "iotarow": iotarow, "ident": ident,
            "rcp": np.ascontiguousarray(rcp_k),
        })

    nc = _get_kernel(S)
    res = run_bass_kernel_spmd(nc, in_maps, core_ids=list(range(N_CORES)))

    out = np.zeros((N_NODES, OUT_C), dtype=np.float32)
    for k in range(N_CORES):
        lo = k * V_CORE
        hi = min(lo + V_CORE, N_NODES)
        out[lo:hi] = res.results[k]["out"][:hi - lo]
    return out


# revision 17
# speedup vs baseline: 1.4201x; 1.0243x over previous
"""NeuralFieldCosmo GNN message-passing kernel for 8 Trainium2 NeuronCores.

Math (see reference):
  per-edge MLP on hood_coords: 3 -> 32 -> LN -> relu -> 32 -> LN -> relu
  -> 1024 -> tanh = per-edge weight matrix w[e, 32, 32]
  oc[e, o] = sum_i features[source[e], i] * w[e, o, i]
  out[v] = mean over edges with target[e] == v  (count-clamped)

Sharding: edges sorted by target; core k owns nodes [1280k, 1280k+1280).
Each core runs 10 node-blocks of 128 nodes; block edges padded to S
subtiles of 128 edges. Features are gathered per subtile by indirect DMA.
Scatter-mean is a one-hot matmul accumulated in PSUM per block; counts
are sharding metadata computed on host as reciprocals. Matmuls run in
fp32r (tf32-like). Output slices are concatenated on host.
"""
import numpy as np

import concourse.bass as bass
import concourse.mybir as mybir
import concourse.tile as tile
from concourse import library_config
from concourse.bass_utils import run_bass_kernel_spmd
from concourse.tile_rust import add_dep_helper

P = 128
IN_C = 32
OUT_C = 32
FC = 32
DIM = 3
OI = IN_C * OUT_C  # 1024
FPAD = 64               # feature rows padded to 64 floats (256B) for dma_gather
LN_EPS = 1e-5
RADIUS = 1.0
N_NODES = 10000
N_CORES = 8
V_CORE = 1280           # nodes per core (10 blocks of 128)
NBLK = V_CORE // P      # 10
DVE_OGROUPS = 26        # einsum mul split: o-groups on DVE, rest GPSIMD
F32 = mybir.dt.float32
F32R = mybir.dt.float32r
I16 = mybir.dt.int16
AX = mybir.AxisListType
OP = mybir.AluOpType
AF = mybir.ActivationFunctionType


# ---------------------------------------------------------------- legalize --
def _legalize_bir_json(bir: bytes) -> bytes:
    """This walrus build allows one sync wait per instruction; move excess
    waits onto NoOps inserted before the offending instruction."""
    import orjson
    j = orjson.loads(bir)
    ctr = 0
    for fn in j.get("functions", []):
        for blk in fn.get("blocks", []):
            out = []
            for ins in blk["instructions"]:
                si = ins.get("sync_info")
                waits = (si or {}).get("on_wait") or []
                if len(waits) > 1:
                    for w in waits[:-1]:
                        out.append({
                            "engine": ins.get("engine"), "ins": [],
                            "name": f"legalize-nop-{ctr}", "opcode": "NoOp",
                            "outs": [],
                            "sync_info": {"on_update": [], "on_wait": [w]},
                        })
                        ctr += 1
                    si["on_wait"] = [waits[-1]]
                out.append(ins)
            blk["instructions"] = out
    return orjson.dumps(j)


def _patch_bass(nc):
    orig = nc.to_json_bytes
    nc.to_json_bytes = lambda: _legalize_bir_json(orig())
    return nc


# ------------------------------------------------------------ device kernel --
def _o_view(ap, og0, og1):
    """[128, og*32 : og1*32] viewed as [128, og1-og0, 32]."""
    return ap[:, og0 * 32:og1 * 32].rearrange("p (o i) -> p o i", i=32)


def _r(ap):
    return ap.bitcast(F32R)


def build_kernel(S: int):
    S_tot = NBLK * S
    E_c = S_tot * P

    nc = bass.Bass()
    xaug_d = nc.dram_tensor("xaug", [DIM + 1, E_c], F32, kind="ExternalInput")
    src_d = nc.dram_tensor("srcT", [P, S_tot], mybir.dt.int32, kind="ExternalInput")
    tgt_d = nc.dram_tensor("tgtloc", [P, S_tot], F32, kind="ExternalInput")
    feat_d = nc.dram_tensor("feat", [N_NODES, IN_C], F32, kind="ExternalInput")
    w0_d = nc.dram_tensor("W0aug", [DIM + 1, FC], F32, kind="ExternalInput")
    w1_d = nc.dram_tensor("W1aug", [FC + 1, FC], F32, kind="ExternalInput")
    w2_d = nc.dram_tensor("W2aug", [FC + 1, OI], F32, kind="ExternalInput")
    ln_d = nc.dram_tensor("lnrow", [1, 4 * FC], F32, kind="ExternalInput")
    lncol_d = nc.dram_tensor("lncol", [FC, 4], F32, kind="ExternalInput")
    iota_d = nc.dram_tensor("iotarow", [1, P], F32, kind="ExternalInput")
    ident_d = nc.dram_tensor("ident", [P, P], F32, kind="ExternalInput")
    rcp_d = nc.dram_tensor("rcp", [P, NBLK], F32, kind="ExternalInput")
    out_d = nc.dram_tensor("out", [V_CORE, OUT_C], F32, kind="ExternalOutput")

    groups = []
    s0 = 0
    while s0 < S_tot:
        g = min(8, S_tot - s0)
        groups.append((s0, g))
        s0 += g

    gpsimd_deps = []   # (from_inst, to_inst) ordering for library correctness

    with tile.TileContext(nc) as tc:
        with (
            tc.tile_pool(name="const", bufs=1) as cp,
            tc.tile_pool(name="arrays", bufs=1) as arr,
            tc.tile_pool(name="work", bufs=4) as wk,
        ):
            # ---------------- preload constants ----------------
            tgt_sb = arr.tile([P, S_tot], F32)
            nc.sync.dma_start(out=tgt_sb[:], in_=tgt_d[:])
            rcp_sb = cp.tile([P, NBLK], F32)
            nc.sync.dma_start(out=rcp_sb[:], in_=rcp_d[:])
            identr = cp.tile([P, P], F32)
            nc.sync.dma_start(out=identr[:], in_=ident_d[:])
            identf = cp.tile([P, P], F32R)
            nc.vector.tensor_copy(out=identf[:], in_=identr[:])

            src_sb = arr.tile([P, S_tot], mybir.dt.int32)
            nc.sync.dma_start(out=src_sb[:], in_=src_d[:])

            def gps_dep(inst):
                return inst

            w0r = cp.tile([DIM + 1, FC], F32)
            nc.sync.dma_start(out=w0r[:], in_=w0_d[:])
            w0f = cp.tile([DIM + 1, FC], F32)
            nc.vector.tensor_copy(out=w0f[:], in_=w0r[:])
            w1r = cp.tile([FC + 1, FC], F32)
            nc.sync.dma_start(out=w1r[:], in_=w1_d[:])
            w1f = cp.tile([FC + 1, FC], F32R)
            nc.vector.tensor_copy(out=w1f[:], in_=w1r[:])
            w2r = cp.tile([FC + 1, OI], F32)
            nc.sync.dma_start(out=w2r[:], in_=w2_d[:])
            w2f = cp.tile([FC + 1, OI], F32R)
            nc.vector.tensor_copy(out=w2f[:], in_=w2r[:])

            lnr = cp.tile([1, 4 * FC], F32)
            nc.sync.dma_start(out=lnr[:], in_=ln_d[:])
            lncol = cp.tile([FC, 4], F32)
            nc.sync.dma_start(out=lncol[:], in_=lncol_d[:])
            iotar = cp.tile([1, P], F32)
            nc.sync.dma_start(out=iotar[:], in_=iota_d[:])
            combo = cp.tile([1, 4 * FC + P], F32)
            nc.vector.tensor_copy(out=combo[:, :4 * FC], in_=lnr[:])
            nc.vector.tensor_copy(out=combo[:, 4 * FC:], in_=iotar[:])
            ones_row = cp.tile([1, P], F32)
            nc.vector.memset(ones_row[:], 1.0)
            eps_t = cp.tile([P, 1], F32)
            nc.vector.memset(eps_t[:], LN_EPS)

            with tc.tile_pool(name="pre_ps", bufs=1, space="PSUM") as pps:
                bc_ps = pps.tile([P, 4 * FC + P], F32, space="PSUM")
                nc.tensor.matmul(out=bc_ps[:], lhsT=ones_row[:], rhs=combo[:],
                                 start=True, stop=True)
                bc = cp.tile([P, 4 * FC + P], F32)
                nc.vector.tensor_copy(out=bc[:], in_=bc_ps[:])
            g0b = bc[:, 0:FC]
            b0b = bc[:, FC:2 * FC]
            g1b = bc[:, 2 * FC:3 * FC]
            b1b = bc[:, 3 * FC:4 * FC]
            iotab = bc[:, 4 * FC:]

            # persistent aug-transpose tiles (row FC is the bias/ones row),
            # ping-ponged to break the copy->matmul serialization chain
            h0nTs, h1nTs = [], []
            for _i in range(3):
                t0_ = cp.tile([FC + 1, P], F32R, tag=f"h0nT{_i}")
                nc.vector.tensor_copy(out=t0_[FC:FC + 1, :], in_=ones_row[:])
                h0nTs.append(t0_)
                t1_ = cp.tile([FC + 1, P], F32R, tag=f"h1nT{_i}")
                nc.vector.tensor_copy(out=t1_[FC:FC + 1, :], in_=ones_row[:])
                h1nTs.append(t1_)

            h0_all = arr.tile([P, FC * S_tot], F32R)
            h1_all = arr.tile([P, FC * S_tot], F32R)
            s_sum = arr.tile([P, S_tot], F32)
            s_sq = arr.tile([P, S_tot], F32)
            mu0 = arr.tile([P, S_tot], F32)
            rv0 = arr.tile([P, S_tot], F32)
            s1sum = arr.tile([P, S_tot], F32)
            s1sq = arr.tile([P, S_tot], F32)
            mu1 = arr.tile([P, S_tot], F32)
            rv1 = arr.tile([P, S_tot], F32)
            msq = arr.tile([P, S_tot], F32)

            # ---------------- phase 1: L0 + LN0 stats ----------------
            with tc.tile_pool(name="ps1", bufs=3, space="PSUM") as ps1:
                for (g0s, gn) in groups:
                    h0ps = ps1.tile([P, FC * 8], F32, space="PSUM", tag="h0g")
                    hsq = wk.tile([P, FC * 8], F32, tag="hsq")
                    xt = wk.tile([DIM + 1, P * 8], F32, tag="xt")
                    nc.sync.dma_start(out=xt[:, :P * gn],
                                      in_=xaug_d[:, P * g0s:P * (g0s + gn)])
                    for sl in range(gn):
                        s = g0s + sl
                        nc.tensor.matmul(
                            out=h0ps[:, FC * sl:FC * (sl + 1)],
                            lhsT=xt[:, P * sl:P * (sl + 1)],
                            rhs=w0f[:], start=True, stop=True)
                        nc.scalar.activation(
                            out=hsq[:, FC * sl:FC * (sl + 1)],
                            in_=h0ps[:, FC * sl:FC * (sl + 1)], func=AF.Square)
                    w = FC * gn
                    nc.vector.tensor_copy(
                        out=h0_all[:, FC * g0s:FC * (g0s + gn)], in_=h0ps[:, :w])
                    nc.vector.tensor_reduce(
                        out=s_sum[:, g0s:g0s + gn],
                        in_=h0ps[:, :w].rearrange("p (a i) -> p a i", i=FC),
                        axis=AX.X, op=OP.add)
                    nc.vector.tensor_reduce(
                        out=s_sq[:, g0s:g0s + gn],
                        in_=hsq[:, :w].rearrange("p (a i) -> p a i", i=FC),
                        axis=AX.X, op=OP.add)

            # ---------------- phase 2: LN small ops (batched) ----------------
            def ln_smalls(ssum, ssq, mu, rv):
                nc.vector.tensor_scalar(out=mu[:], in0=ssum[:], scalar1=1.0 / FC,
                                        scalar2=None, op0=OP.mult)
                nc.vector.tensor_scalar(out=ssq[:], in0=ssq[:], scalar1=1.0 / FC,
                                        scalar2=None, op0=OP.mult)
                nc.vector.tensor_tensor(out=msq[:], in0=mu[:], in1=mu[:],
                                        op=OP.mult)
                nc.vector.tensor_tensor(out=ssq[:], in0=ssq[:], in1=msq[:],
                                        op=OP.subtract)
                nc.scalar.activation(out=ssq[:], in_=ssq[:], func=AF.Sqrt,
                                     bias=eps_t[:, 0:1])
                nc.vector.reciprocal(out=rv[:], in_=ssq[:])

            ln_smalls(s_sum, s_sq, mu0, rv0)

            # ---------------- phase 3: normalize (in place) ----------------
            def normalize(h_all, mu, rv, gb, bb):
                for gi, (g0s, gn) in enumerate(groups):
                    use_gps = gi % 3 == 2
                    eng = nc.gpsimd if use_gps else nc.vector
                    hv = h_all[:, FC * g0s:FC * (g0s + gn)].rearrange(
                        "p (a i) -> p a i", i=FC)
                    mub = mu[:, g0s:g0s + gn].rearrange(
                        "p (a i) -> p a i", i=1).to_broadcast([P, gn, FC])
                    rvb = rv[:, g0s:g0s + gn].rearrange(
                        "p (a i) -> p a i", i=1).to_broadcast([P, gn, FC])
                    gbt = gb.rearrange("p (a i) -> p a i", a=1).to_broadcast(
                        [P, gn, FC])
                    bbt = bb.rearrange("p (a i) -> p a i", a=1).to_broadcast(
                        [P, gn, FC])
                    ops = [
                        eng.tensor_tensor(out=hv, in0=hv, in1=mub, op=OP.subtract),
                        eng.tensor_tensor(out=hv, in0=hv, in1=rvb, op=OP.mult),
                    ]
                    if use_gps:
                        for o in ops:
                            gps_dep(o)

            normalize(h0_all, mu0, rv0, g0b, b0b)

            # ---------------- phase 4: transpose + L1 + LN1 stats ----------------
            with tc.tile_pool(name="ps4", bufs=3, space="PSUM") as ps4:
                for (g0s, gn) in groups:
                    h1ps = ps4.tile([P, FC * 8], F32, space="PSUM", tag="h1g")
                    hsq = wk.tile([P, FC * 8], F32, tag="hsq")
                    for sl in range(gn):
                        s = g0s + sl
                        t0ps = ps4.tile([FC, P], F32R, space="PSUM", tag="t0")
                        nc.tensor.transpose(
                            out=t0ps[:], in_=h0_all[:, FC * s:FC * (s + 1)],
                            identity=identf[:])
                        h0nT = h0nTs[s % 3]
                        nc.scalar.activation(
                            out=h0nT[:FC, :], in_=t0ps[:], func=AF.Relu,
                            scale=lncol[:, 0:1], bias=lncol[:, 1:2])
                        nc.tensor.matmul(
                            out=h1ps[:, FC * sl:FC * (sl + 1)],
                            lhsT=h0nT[:], rhs=w1f[:], start=True, stop=True)
                        nc.scalar.activation(
                            out=hsq[:, FC * sl:FC * (sl + 1)],
                            in_=h1ps[:, FC * sl:FC * (sl + 1)], func=AF.Square)
                    w = FC * gn
                    nc.vector.tensor_copy(
                        out=h1_all[:, FC * g0s:FC * (g0s + gn)], in_=h1ps[:, :w])
                    nc.vector.tensor_reduce(
                        out=s1sum[:, g0s:g0s + gn],
                        in_=h1ps[:, :w].rearrange("p (a i) -> p a i", i=FC),
                        axis=AX.X, op=OP.add)
                    nc.vector.tensor_reduce(
                        out=s1sq[:, g0s:g0s + gn],
                        in_=hsq[:, :w].rearrange("p (a i) -> p a i", i=FC),
                        axis=AX.X, op=OP.add)

            # ---------------- phases 5+6: LN1 ----------------
            ln_smalls(s1sum, s1sq, mu1, rv1)
            normalize(h1_all, mu1, rv1, g1b, b1b)

            # ---------------- phase 7: L2 + tanh + mul + scatter ----------------
            with tc.tile_pool(name="ps7", bufs=1, space="PSUM") as psA, \
                 tc.tile_pool(name="ps7b", bufs=2, space="PSUM") as psB:
                for b in range(NBLK):
                    acc = psA.tile([P, OI], F32, space="PSUM", tag="acc")
                    for si in range(S):
                        s = b * S + si
                        oh = wk.tile([P, P], F32R, tag="oh")
                        nc.vector.tensor_tensor(
                            out=oh[:],
                            in0=tgt_sb[:, s:s + 1].to_broadcast([P, P]),
                            in1=iotab, op=OP.is_equal)
                        t1ps = psB.tile([FC, P], F32R, space="PSUM", tag="t1")
                        nc.tensor.transpose(
                            out=t1ps[:], in_=h1_all[:, FC * s:FC * (s + 1)],
                            identity=identf[:])
                        h1nT = h1nTs[s % 3]
                        nc.scalar.activation(
                            out=h1nT[:FC, :], in_=t1ps[:], func=AF.Relu,
                            scale=lncol[:, 2:3], bias=lncol[:, 3:4])
                        zps = psB.tile([P, OI], F32, space="PSUM", tag="z")
                        nc.tensor.matmul(out=zps[:, 0:512], lhsT=h1nT[:],
                                         rhs=w2f[:, 0:512], start=True, stop=True)
                        nc.tensor.matmul(out=zps[:, 512:OI], lhsT=h1nT[:],
                                         rhs=w2f[:, 512:OI], start=True, stop=True)
                        t = wk.tile([P, OI], F32, tag="t")
                        nc.scalar.activation(out=t[:], in_=zps[:], func=AF.Tanh)
                        f = wk.tile([P, IN_C], F32, tag="f")
                        nc.gpsimd.indirect_dma_start(
                            out=f[:], out_offset=None, in_=feat_d[:],
                            in_offset=bass.IndirectOffsetOnAxis(
                                ap=src_sb[:, s:s + 1], axis=0))
                        prod = wk.tile([P, OI], F32R, tag="prod")
                        fb = f[:].rearrange("p (a i) -> p a i", a=1)
                        dg = DVE_OGROUPS
                        nc.vector.tensor_tensor(
                            out=_o_view(prod, 0, dg), in0=_o_view(t, 0, dg),
                            in1=fb.to_broadcast([P, dg, IN_C]), op=OP.mult)
                        gps_dep(nc.gpsimd.tensor_tensor(
                            out=_o_view(prod, dg, OUT_C), in0=_o_view(t, dg, OUT_C),
                            in1=fb.to_broadcast([P, OUT_C - dg, IN_C]), op=OP.mult))
                        nc.tensor.matmul(out=acc[:, 0:512], lhsT=oh[:],
                                         rhs=prod[:, 0:512], start=(si == 0),
                                         stop=(si == S - 1))
                        nc.tensor.matmul(out=acc[:, 512:OI], lhsT=oh[:],
                                         rhs=prod[:, 512:OI], start=(si == 0),
                                         stop=(si == S - 1))
                    ocv = wk.tile([P, OUT_C], F32, tag="ocv")
                    nc.vector.tensor_reduce(
                        out=ocv[:], in_=acc[:].rearrange("p (o i) -> p o i", i=IN_C),
                        axis=AX.X, op=OP.add)
                    osb = wk.tile([P, OUT_C], F32, tag="osb")
                    nc.vector.tensor_scalar(
                        out=osb[:], in0=ocv[:], scalar1=rcp_sb[:, b:b + 1],
                        scalar2=None, op0=OP.mult)
                    nc.sync.dma_start(out=out_d[P * b:P * (b + 1), :], in_=osb[:])

        for frm, to in gpsimd_deps:
            # add_dep_helper(A, B) == "A waits on B"
            add_dep_helper(to.ins, frm.ins, sync=False, reason="gpsimd library order")

    _patch_bass(nc)
    return nc


# ------------------------------------------------------------- host wrapper --
_cache = {}


def _get_kernel(S):
    if S not in _cache:
        _cache[S] = build_kernel(S)
    return _cache[S]


def kernel(features, hood_coords, source, target,
           W0, b0, g0, beta0, W1, b1, g1, beta1, W2, b2):
    features = np.asarray(features, dtype=np.float32)
    hood = np.asarray(hood_coords, dtype=np.float32)
    source = np.asarray(source).astype(np.int64)
    target = np.asarray(target).astype(np.int64)
    W0 = np.asarray(W0, dtype=np.float32)
    W1 = np.asarray(W1, dtype=np.float32)
    W2 = np.asarray(W2, dtype=np.float32)
    b0 = np.asarray(b0, dtype=np.float32)
    b1 = np.asarray(b1, dtype=np.float32)
    b2 = np.asarray(b2, dtype=np.float32)
    g0 = np.asarray(g0, dtype=np.float32)
    g1 = np.asarray(g1, dtype=np.float32)
    beta0 = np.asarray(beta0, dtype=np.float32)
    beta1 = np.asarray(beta1, dtype=np.float32)

    perm = np.argsort(target, kind="stable")
    tgt_s = target[perm]
    src_s = source[perm]
    hood_s = hood[perm]

    blk_starts = np.searchsorted(tgt_s, np.arange(0, N_CORES * V_CORE + 1, P))
    nseg = np.diff(blk_starts)
    S = max(1, int(np.ceil(nseg.max() / P)))
    S_tot = NBLK * S
    E_c = S_tot * P

    counts = np.bincount(target.astype(np.int64),
                         minlength=N_CORES * V_CORE).astype(np.float32)
    rcp_full = 1.0 / np.maximum(counts, 1.0)


    W0aug = np.vstack([W0 / RADIUS, b0[None, :]]).astype(np.float32)
    W1aug = np.vstack([W1, b1[None, :]]).astype(np.float32)
    W2aug = np.vstack([W2, b2[None, :]]).astype(np.float32)
    lnrow = np.concatenate([g0, beta0, g1, beta1])[None, :].astype(np.float32)
    lncol = np.stack([g0, beta0, g1, beta1], axis=1).astype(np.float32)
    iotarow = np.arange(P, dtype=np.float32)[None, :]
    ident = np.eye(P, dtype=np.float32)

    in_maps = []
    for k in range(N_CORES):
        xflat = np.zeros((E_c, DIM + 1), dtype=np.float32)
        xflat[:, DIM] = 1.0
        srcflat = np.zeros(E_c, dtype=np.int32)
        tgtflat = np.full(E_c, -1.0, dtype=np.float32)
        for b in range(NBLK):
            gi = k * NBLK + b
            e0, e1 = blk_starts[gi], blk_starts[gi + 1]
            n = e1 - e0
            if n == 0:
                continue
            p0 = b * S * P
            xflat[p0:p0 + n, :DIM] = hood_s[e0:e1]
            srcflat[p0:p0 + n] = src_s[e0:e1].astype(np.int32)
            tgtflat[p0:p0 + n] = (tgt_s[e0:e1] - (k * V_CORE + b * P)).astype(np.float32)
        rcp_k = rcp_full[k * V_CORE:(k + 1) * V_CORE].reshape(NBLK, P).T.copy()
        in_maps.append({
            "xaug": np.ascontiguousarray(xflat.T),
            "srcT": np.ascontiguousarray(srcflat.reshape(S_tot, P).T),
            "tgtloc": np.ascontiguousarray(tgtflat.reshape(S_tot, P).T),
            "feat": features,
            "W0aug": W0aug, "W1aug": W1aug, "W2aug": W2aug,
            "lnrow": lnrow, "lncol": lncol, "iotarow": iotarow, "ident": ident,
            "rcp": np.ascontiguousarray(rcp_k),
        })

    nc = _get_kernel(S)
    res = run_bass_kernel_spmd(nc, in_maps, core_ids=list(range(N_CORES)))

    out = np.zeros((N_NODES, OUT_C), dtype=np.float32)
    for k in range(N_CORES):
        lo = k * V_CORE
        hi = min(lo + V_CORE, N_NODES)
        out[lo:hi] = res.results[k]["out"][:hi - lo]
    return out
